# revision 12
# baseline (speedup 1.0000x reference)
"""ArcticDecoderLayer on 8 TRN2 NeuronCores.

Sharding:
  - tokens: zigzag block-parallel (core i owns 128-token blocks {i, 15-i});
    attention, wo, residual MLP are token-parallel (weights replicated).
  - MoE: expert-parallel, 2 experts/core; routing/top-2 + token gather/scatter
    done host-side (part of shard/unshard), expert GEMMs on device.
  - One AllGather (K^T feature-major + V token-major, bf16) is the only
    collective; causal masking is data-driven (per-core exp-bias columns) so
    the SPMD graph is identical on all cores.

All matmuls in bf16 (fp32 PE matmul is 4x slower); accumulation in f32 PSUM.
Weight streaming uses few large multi-k-chunk DMAs (3D APs) split across the
two HWDGE rings (sync + scalar) to keep sequencer issue cost off the critical
path.
"""
import numpy as np
import ml_dtypes

import concourse.bacc as bacc
import concourse.tile as tile
import concourse.mybir as mybir
from concourse.bass_utils import run_bass_kernel_spmd

F32 = mybir.dt.float32
BF16 = mybir.dt.bfloat16
AF = mybir.ActivationFunctionType

H = 2048
NH = 16
NKV = 4
HD = 128
HALF = 64
I = 1024
E = 16
TOPK = 2
T = 2048
EPS = 1e-5
THETA = 10000.0
NC_ = 8
BLK = 128
NBLK = 16
TPC = 256  # tokens per core
EPC = 2  # experts per core
SCALE = HD ** -0.5
NEG = -30000.0
KQ = H // BLK  # 16
MI = I // BLK  # 8

TRACE = False
LAST_RESULT = None
_CACHE = {}

bf = lambda a: np.ascontiguousarray(np.asarray(a).astype(ml_dtypes.bfloat16))
f32 = lambda a: np.ascontiguousarray(a, dtype=np.float32)


def _slot(kb):
    c = min(kb, NBLK - 1 - kb)
    return 2 * c + (0 if kb < NC_ else 1)


def _build(cap):
    nc = bacc.Bacc("TRN2", target_bir_lowering=False, debug=False, num_devices=NC_)

    din = lambda name, shape, dt=BF16: nc.dram_tensor(name, shape, dt, kind="ExternalInput")
    xT_bf_d = din("xT_bf", [H, TPC])
    xT32_d = din("xT32", [H, TPC], F32)
    cos_d = din("cos_s", [HALF, TPC], F32)
    sin_d = din("sin_s", [HALF, TPC], F32)
    scol_d = din("s_col", [BLK, 2], F32)
    bias_d = din("bias", [2, NBLK, BLK], F32)
    tri_d = din("tri", [BLK, BLK], F32)
    ident_d = din("ident", [BLK, BLK])
    wqkv_d = din("wqkv", [H, NH * HD + 2 * NKV * HD])
    wo_d = din("wo", [NH * HD, H])
    w13_d = din("w13", [H, 2 * H])  # host-interleaved: chunk 2p=g_p, 2p+1=u_p
    w2_d = din("w2", [H, H])
    wsT_d = din("wsT", [EPC, H, 2 * I])  # host-interleaved g/u pairs
    w2sT_d = din("w2sT", [EPC, I, H])
    xg_d = din("xgT", [EPC, H, cap])
    ew_d = din("ew", [EPC, BLK, cap], F32)

    res_out_d = nc.dram_tensor("res_out", [H, TPC], F32, kind="ExternalOutput")
    moe_out_d = nc.dram_tensor("moe_out", [EPC, H, cap], F32, kind="ExternalOutput")

    KG = 4  # contraction chunks per weight-stream DMA
    SW = 8  # m-chunks per sweep for TN=256 GEMMs (4 paired psum banks)

    with tile.TileContext(nc) as tc:
        with (
            tc.tile_pool(name="res", bufs=1) as res,
            tc.tile_pool(name="stream", bufs=2) as stream,
            tc.tile_pool(name="small", bufs=3) as small,
            tc.tile_pool(name="outp", bufs=3) as outp,
            tc.tile_pool(name="acc", bufs=4, space="PSUM") as acc,
            tc.tile_pool(name="macc", bufs=4, space="PSUM") as macc,
            tc.tile_pool(name="dram", bufs=1, space="DRAM") as dram,
        ):
            eng_rr = [nc.sync, nc.scalar]

            # ---------------- resident loads ----------------
            xT_sb = res.tile([BLK, KQ * TPC], BF16, tag="xT")
            xT32_sb = res.tile([BLK, KQ * TPC], F32, tag="xT32")
            nc.sync.dma_start(xT_sb[:].rearrange("p (k t) -> p k t", k=KQ),
                              xT_bf_d.ap().rearrange("(k p) t -> p k t", p=BLK))
            nc.scalar.dma_start(xT32_sb[:].rearrange("p (k t) -> p k t", k=KQ),
                                xT32_d.ap().rearrange("(k p) t -> p k t", p=BLK))
            cos_sb = res.tile([HALF, TPC], F32, tag="cos")
            sin_sb = res.tile([HALF, TPC], F32, tag="sin")
            nc.sync.dma_start(cos_sb[:], cos_d[:])
            nc.sync.dma_start(sin_sb[:], sin_d[:])
            scol_sb = res.tile([BLK, 2], F32, tag="scol")
            nc.sync.dma_start(scol_sb[:], scol_d[:])
            bias_sb = res.tile([BLK, 2 * NBLK], F32, tag="bias")
            nc.sync.dma_start(bias_sb[:], bias_d.ap().rearrange("a k p -> p (a k)"))
            tri_sb = res.tile([BLK, BLK], F32, tag="tri")
            nc.sync.dma_start(tri_sb[:], tri_d[:])
            ident_sb = res.tile([BLK, BLK], BF16, tag="ident")
            nc.sync.dma_start(ident_sb[:], ident_d[:])
            ew_sb = res.tile([BLK, EPC * cap], F32, tag="ew")
            for e in range(EPC):
                nc.scalar.dma_start(ew_sb[:, e * cap:(e + 1) * cap], ew_d[e])
            xg_sb = res.tile([BLK, EPC * KQ * cap], BF16, tag="xg")
            for e in range(EPC):
                nc.scalar.dma_start(
                    xg_sb[:, e * KQ * cap:(e + 1) * KQ * cap].rearrange("p (k t) -> p k t", k=KQ),
                    xg_d[e].rearrange("(k p) t -> p k t", p=BLK))

            ones_bf = res.tile([BLK, 1], BF16, tag="onesb")
            nc.vector.memset(ones_bf[:], 1.0)
            ones_row = res.tile([1, BLK], F32, tag="onesr")
            nc.vector.memset(ones_row[:], 1.0)
            ones_cf = res.tile([BLK, 1], F32, tag="onescf")
            nc.vector.memset(ones_cf[:], 1.0)

            q_sb = res.tile([BLK, NH * TPC], BF16, tag="q")
            k_sb = res.tile([BLK, NKV * TPC], BF16, tag="k")
            v_sb = res.tile([BLK, 2 * NKV * HD], BF16, tag="v")
            attnT_sb = res.tile([BLK, NH * TPC], BF16, tag="attnT")
            resid_sb = res.tile([BLK, KQ * TPC], F32, tag="resid")
            h2_sb = res.tile([BLK, KQ * TPC], BF16, tag="h2")
            gu_sb = res.tile([BLK, KQ * TPC], BF16, tag="gu")
            hm_sb = res.tile([BLK, EPC * MI * cap], BF16, tag="hm")
            kag_sb = res.tile([BLK, NBLK * NKV * BLK], BF16, tag="kag")
            vag_sb = res.tile([BLK, NBLK * NKV * BLK], BF16, tag="vag")

            # ============ generic streamed GEMM sweep ============
            # out chunk m = w[:, m*128:(m+1)*128].T @ rhs ; accumulate over kcnt
            # contraction chunks. consume(m, ps_ap) or consume_pair(j, ps, ms).
            def gemm(w_src, mcnt, kcnt, rhs_fn, pool, tn, consume, sweep, dma_eng, tag):
                pair = 2 * tn <= 512
                for s0 in range(0, mcnt, sweep):
                    ms = list(range(s0, min(s0 + sweep, mcnt)))
                    mw = len(ms)
                    if pair:
                        nt = (mw + 1) // 2
                        pts = [pool.tile([BLK, 2 * tn], F32, tag=tag,
                                         name=f"pt{j}") for j in range(nt)]
                        paps = [pts[j // 2][:, (j % 2) * tn:(j % 2 + 1) * tn]
                                for j in range(mw)]
                    else:
                        pts = [pool.tile([BLK, tn], F32, tag=tag,
                                         name=f"pt{j}") for j in range(mw)]
                        paps = [pts[j][:] for j in range(mw)]
                    for kg0 in range(0, kcnt, KG):
                        kgn = min(KG, kcnt - kg0)
                        wt = stream.tile([BLK, KG * SW * BLK], BF16, tag="wt")
                        dma_eng[(kg0 // KG) % 2].dma_start(
                            wt[:, :kgn * mw * BLK].rearrange("p (k c) -> p k c", k=kgn),
                            w_src(kg0, kgn, ms[0] * BLK, (ms[-1] + 1) * BLK))
                        for kl in range(kgn):
                            k = kg0 + kl
                            for j in range(mw):
                                nc.tensor.matmul(
                                    paps[j], wt[:, (kl * mw + j) * BLK:(kl * mw + j + 1) * BLK],
                                    rhs_fn(k), start=(k == 0), stop=(k == kcnt - 1))
                    consume(ms, paps, pts)

            # ---------------- QKV projection (feature-major out) ----------------
            def rope_chunk(ps, dst, col0):
                t1 = small.tile([HALF, TPC], F32, tag="r1")
                t2 = small.tile([HALF, TPC], F32, tag="r2")
                nc.vector.tensor_mul(t1[:], ps[0:HALF, :], cos_sb[:])
                nc.vector.tensor_mul(t2[:], ps[HALF:BLK, :], sin_sb[:])
                nc.vector.tensor_sub(dst[0:HALF, col0:col0 + TPC], t1[:], t2[:])
                t3 = small.tile([HALF, TPC], F32, tag="r1")
                t4 = small.tile([HALF, TPC], F32, tag="r2")
                nc.vector.tensor_mul(t3[:], ps[HALF:BLK, :], cos_sb[:])
                nc.vector.tensor_mul(t4[:], ps[0:HALF, :], sin_sb[:])
                nc.vector.tensor_add(dst[HALF:BLK, col0:col0 + TPC], t3[:], t4[:])

            def qkv_consume(ms, paps, pts):
                for m, ps in zip(ms, paps):
                    if m < NH:
                        rope_chunk(ps, q_sb, m * TPC)
                    elif m < NH + NKV:
                        rope_chunk(ps, k_sb, (m - NH) * TPC)
                    else:
                        kvh = m - NH - NKV
                        vtmp = small.tile([BLK, TPC], BF16, tag="vtmp")
                        nc.scalar.activation(vtmp[:], ps, AF.Copy)
                        for tb in range(2):
                            pt = macc.tile([BLK, BLK], BF16, tag="macct")
                            nc.tensor.transpose(pt[:], vtmp[:, tb * BLK:(tb + 1) * BLK], ident_sb[:])
                            nc.vector.tensor_scalar_mul(
                                v_sb[:, (tb * NKV + kvh) * BLK:(tb * NKV + kvh + 1) * BLK],
                                pt[:], scol_sb[:, tb:tb + 1])

            wqkv_src = lambda kg0, kgn, c0, c1: wqkv_d[kg0 * BLK:(kg0 + kgn) * BLK, c0:c1] \
                .rearrange("(k p) c -> p k c", p=BLK)
            gemm(wqkv_src, NH + 2 * NKV, KQ,
                 lambda k: xT_sb[:, k * TPC:(k + 1) * TPC], acc, TPC, qkv_consume,
                 SW, eng_rr, "acct")

            # ---------------- KV AllGather ----------------
            KSZ = NKV * BLK * TPC  # 131072 elems
            kv_local = dram.tile([2, KSZ], BF16)
            kv_ag = dram.tile([NC_, 2, KSZ], BF16, addr_space="Shared")
            nc.sync.dma_start(
                kv_local[0, :].rearrange("(h d t) -> d h t", h=NKV, d=BLK),
                k_sb[:].rearrange("d (h t) -> d h t", h=NKV))
            nc.sync.dma_start(
                kv_local[1, :].rearrange("(b h t d) -> t b h d", b=2, h=NKV, t=BLK),
                v_sb[:].rearrange("t (b h d) -> t b h d", b=2, h=NKV))
            nc.gpsimd.collective_compute(
                "AllGather", mybir.AluOpType.bypass,
                replica_groups=[list(range(NC_))],
                ins=[kv_local[:]], outs=[kv_ag[:]])
            # unpack in AG order (slot = 2c+sub); attention maps kb -> slot.
            for sub in range(2):
                nc.sync.dma_start(
                    kag_sb[:].rearrange("d (c s h t) -> s d c h t", c=NC_, s=2, h=NKV)[sub],
                    kv_ag[:, 0, :].rearrange("c (h d s t) -> s d c h t", h=NKV, d=BLK, s=2)[sub])
                nc.scalar.dma_start(
                    vag_sb[:].rearrange("t (c s h dd) -> s t c h dd", c=NC_, s=2, h=NKV)[sub],
                    kv_ag[:, 1, :].rearrange("c (s h t dd) -> s t c h dd", s=2, h=NKV, t=BLK)[sub])

            # ---------------- MoE (overlaps AG + attention) ----------------
            for e in range(EPC):
                rhs_e = lambda k, e=e: xg_sb[:, (e * KQ + k) * cap:(e * KQ + k + 1) * cap]
                pend_g = {}

                def gu_consume(ms, paps, pts, e=e, pend_g=pend_g):
                    for m, ps, pt in zip(ms, paps, pts):
                        if m % 2 == 0:
                            pend_g[m] = (ps, pt)
                        else:
                            gps, _ = pend_g.pop(m - 1)
                            sg = small.tile([BLK, cap], BF16, tag="sg")
                            nc.scalar.activation(sg[:], gps, AF.Silu)
                            p = m // 2
                            nc.vector.tensor_mul(
                                hm_sb[:, (e * MI + p) * cap:(e * MI + p + 1) * cap],
                                sg[:], ps)

                ws_src = lambda kg0, kgn, c0, c1, e=e: wsT_d[e, kg0 * BLK:(kg0 + kgn) * BLK, c0:c1] \
                    .rearrange("(k p) c -> p k c", p=BLK)
                gemm(ws_src, 2 * MI, KQ, rhs_e, macc, cap, gu_consume, 4, eng_rr, "macct")

                def w2s_consume(ms, paps, pts, e=e):
                    for m, ps in zip(ms, paps):
                        mo = outp.tile([BLK, cap], F32, tag="mo")
                        nc.vector.tensor_mul(mo[:], ps, ew_sb[:, e * cap:(e + 1) * cap])
                        nc.scalar.dma_start(moe_out_d[e, m * BLK:(m + 1) * BLK, :], mo[:])

                w2s_src = lambda kg0, kgn, c0, c1, e=e: w2sT_d[e, kg0 * BLK:(kg0 + kgn) * BLK, c0:c1] \
                    .rearrange("(k p) c -> p k c", p=BLK)
                gemm(w2s_src, KQ, MI,
                     lambda k, e=e: hm_sb[:, (e * MI + k) * cap:(e * MI + k + 1) * cap],
                     macc, cap, w2s_consume, 4, eng_rr, "macct")

            # ---------------- attention ----------------
            for h in range(NH):
                kvh = h // (NH // NKV)
                for qb in range(2):
                    nkb = 8 if qb == 0 else 16
                    qv = q_sb[:, h * TPC + qb * BLK: h * TPC + (qb + 1) * BLK]
                    aps = acc.tile([BLK, BLK], F32, tag="acct")
                    dps = acc.tile([1, BLK], F32, tag="acct")
                    # diagonal (own tokens, static tri mask)
                    sps = acc.tile([BLK, BLK], F32, tag="acct")
                    nc.tensor.matmul(
                        sps[:], k_sb[:, kvh * TPC + qb * BLK: kvh * TPC + (qb + 1) * BLK],
                        qv, start=True, stop=True)
                    stmp = small.tile([BLK, BLK], F32, tag="stmp")
                    nc.vector.tensor_add(stmp[:], sps[:], tri_sb[:])
                    pd = small.tile([BLK, BLK], BF16, tag="pd")
                    nc.scalar.activation(pd[:], stmp[:], AF.Exp, scale=SCALE)
                    nc.tensor.matmul(aps[:], v_sb[:, (qb * NKV + kvh) * BLK:(qb * NKV + kvh + 1) * BLK],
                                     pd[:], start=True, stop=False)
                    nc.tensor.matmul(dps[:], ones_bf[:], pd[:], start=True, stop=False)
                    # dense blocks (AG buffer, slot order), mask via bias column
                    for kb in range(nkb):
                        sl = _slot(kb)
                        sps2 = acc.tile([BLK, BLK], F32, tag="acct")
                        nc.tensor.matmul(
                            sps2[:], kag_sb[:, (sl * NKV + kvh) * BLK:(sl * NKV + kvh + 1) * BLK],
                            qv, start=True, stop=True)
                        pdn = small.tile([BLK, BLK], BF16, tag="pd")
                        nc.scalar.activation(
                            pdn[:], sps2[:], AF.Exp, scale=SCALE,
                            bias=bias_sb[:, qb * NBLK + kb: qb * NBLK + kb + 1])
                        last = kb == nkb - 1
                        nc.tensor.matmul(aps[:], vag_sb[:, (sl * NKV + kvh) * BLK:(sl * NKV + kvh + 1) * BLK],
                                         pdn[:], start=False, stop=last)
                        nc.tensor.matmul(dps[:], ones_bf[:], pdn[:], start=False, stop=last)
                    # normalize: recip(denom) -> K=1 broadcast matmul -> mul
                    rec = small.tile([1, BLK], F32, tag="rec")
                    nc.vector.reciprocal(rec[:], dps[:])
                    bcp = acc.tile([BLK, BLK], F32, tag="acct")
                    nc.tensor.matmul(bcp[:], ones_row[:], rec[:], start=True, stop=True)
                    bcs = small.tile([BLK, BLK], F32, tag="bcs")
                    nc.scalar.activation(bcs[:], bcp[:], AF.Copy)
                    nc.vector.tensor_mul(
                        attnT_sb[:, h * TPC + qb * BLK: h * TPC + (qb + 1) * BLK],
                        aps[:], bcs[:])

            # ---------------- wo + residual ----------------
            def wo_consume(ms, paps, pts):
                for jt, pt in enumerate(pts):
                    m0 = ms[2 * jt]
                    w = pt.shape[1]
                    nc.vector.tensor_add(resid_sb[:, m0 * TPC: m0 * TPC + w],
                                         pt[:], xT32_sb[:, m0 * TPC: m0 * TPC + w])

            wo_src = lambda kg0, kgn, c0, c1: wo_d[kg0 * BLK:(kg0 + kgn) * BLK, c0:c1] \
                .rearrange("(k p) c -> p k c", p=BLK)
            gemm(wo_src, KQ, KQ, lambda k: attnT_sb[:, k * TPC:(k + 1) * TPC],
                 acc, TPC, wo_consume, SW, eng_rr, "acct")

            # ---------------- residual MLP norm scale ----------------
            ssq = acc.tile([1, TPC], F32, tag="acct")
            for k in range(KQ):
                sq = small.tile([BLK, TPC], F32, tag="sq")
                nc.vector.tensor_mul(sq[:], resid_sb[:, k * TPC:(k + 1) * TPC],
                                     resid_sb[:, k * TPC:(k + 1) * TPC])
                nc.tensor.matmul(ssq[:], ones_cf[:], sq[:],
                                 start=(k == 0), stop=(k == KQ - 1))
            vtmp2 = small.tile([1, TPC], F32, tag="vt")
            nc.vector.tensor_scalar(vtmp2[:], ssq[:], 1.0 / H, EPS,
                                    mybir.AluOpType.mult, mybir.AluOpType.add)
            st = small.tile([1, TPC], F32, tag="vt2")
            nc.scalar.activation(st[:], vtmp2[:], AF.Sqrt)
            s2r = small.tile([1, TPC], F32, tag="vt3")
            nc.vector.reciprocal(s2r[:], st[:])
            s2p = acc.tile([BLK, TPC], F32, tag="acct")
            nc.tensor.matmul(s2p[:], ones_row[:], s2r[:], start=True, stop=True)
            s2s = small.tile([BLK, TPC], F32, tag="s2s")
            nc.scalar.activation(s2s[:], s2p[:], AF.Copy)
            for k in range(KQ):
                nc.vector.tensor_mul(h2_sb[:, k * TPC:(k + 1) * TPC],
                                     resid_sb[:, k * TPC:(k + 1) * TPC], s2s[:])

            # ---------------- w13 (interleaved g/u) + silu_and_mul ----------------
            def w13_consume(ms, paps, pts):
                for jt, pt in enumerate(pts):
                    p = ms[2 * jt] // 2
                    sg = small.tile([BLK, TPC], BF16, tag="sg13")
                    nc.scalar.activation(sg[:], pt[:, 0:TPC], AF.Silu)
                    nc.vector.tensor_mul(gu_sb[:, p * TPC:(p + 1) * TPC],
                                         sg[:], pt[:, TPC:2 * TPC])

            w13_src = lambda kg0, kgn, c0, c1: w13_d[kg0 * BLK:(kg0 + kgn) * BLK, c0:c1] \
                .rearrange("(k p) c -> p k c", p=BLK)
            gemm(w13_src, 2 * KQ, KQ, lambda k: h2_sb[:, k * TPC:(k + 1) * TPC],
                 acc, TPC, w13_consume, SW, eng_rr, "acct")

            # ---------------- w2 + final out ----------------
            def w2_consume(ms, paps, pts):
                for jt, pt in enumerate(pts):
                    m0 = ms[2 * jt]
                    w = pt.shape[1]
                    fo = outp.tile([BLK, 2 * TPC], F32, tag="fo")
                    nc.vector.tensor_add(fo[:, :w], pt[:], resid_sb[:, m0 * TPC:m0 * TPC + w])
                    nc.sync.dma_start(
                        res_out_d.ap().rearrange("(m p) t -> p m t", p=BLK)[:, m0:m0 + w // TPC],
                        fo[:, :w].rearrange("p (m t) -> p m t", t=TPC))

            w2_src = lambda kg0, kgn, c0, c1: w2_d[kg0 * BLK:(kg0 + kgn) * BLK, c0:c1] \
                .rearrange("(k p) c -> p k c", p=BLK)
            gemm(w2_src, KQ, KQ, lambda k: gu_sb[:, k * TPC:(k + 1) * TPC],
                 acc, TPC, w2_consume, SW, eng_rr, "acct")

    nc.compile()
    return nc


def _interleave_cols(w, half):
    # [rows, 2*half] -> column chunks reordered so chunk 2p=g_p, 2p+1=u_p
    rows = w.shape[0]
    g = w[:, :half].reshape(rows, half // BLK, BLK)
    u = w[:, half:].reshape(rows, half // BLK, BLK)
    out = np.empty((rows, 2 * (half // BLK), BLK), w.dtype)
    out[:, 0::2] = g
    out[:, 1::2] = u
    return out.reshape(rows, 2 * half // BLK * BLK)


def kernel(**inputs):
    global LAST_RESULT
    hidden = f32(inputs["hidden_states"])
    positions = np.asarray(inputs["positions"]).astype(np.float32)
    ln_in_w = f32(inputs["ln_in_w"])
    ln_post_w = f32(inputs["ln_post_w"])
    ln_res_w = f32(inputs["ln_res_w"])
    wqkv = f32(inputs["wqkv"])
    wo = f32(inputs["wo"])
    res_w13 = f32(inputs["res_w13"])
    res_w2 = f32(inputs["res_w2"])
    gate_w = f32(inputs["gate_w"])
    ws = f32(inputs["ws"])
    w2s = f32(inputs["w2s"])

    # ---- host prep (sharding) ----
    s = 1.0 / np.sqrt(np.mean(hidden * hidden, axis=1) + EPS)  # [T]
    x_norm = hidden * s[:, None]

    logits = (x_norm * ln_post_w) @ gate_w
    pr = np.exp(logits - logits.max(-1, keepdims=True))
    pr /= pr.sum(-1, keepdims=True)
    topi = np.argsort(-pr, axis=-1, kind="stable")[:, :TOPK]
    topw = np.take_along_axis(pr, topi, axis=-1)
    topw /= topw.sum(-1, keepdims=True)
    tok_lists = [np.where((topi == e).any(-1))[0] for e in range(E)]
    wts = [np.sum(np.where(topi[tl] == e, topw[tl], 0.0), -1).astype(np.float32)
           for e, tl in zip(range(E), tok_lists)]
    cap = max(128, -(-max(len(t) for t in tok_lists) // 64) * 64)
    assert cap <= 512, cap

    if cap not in _CACHE:
        _CACHE[cap] = _build(cap)
    nc = _CACHE[cap]

    inv_freq = 1.0 / (THETA ** (np.arange(0, HD, 2, dtype=np.float32) / HD))
    ang = positions[:, None] * inv_freq
    cos_t, sin_t = np.cos(ang), np.sin(ang)

    tri = np.where(np.arange(BLK)[None, :] >= np.arange(BLK)[:, None], 0.0, NEG).astype(np.float32)
    ident = np.eye(BLK, dtype=np.float32)

    wqkv_f = wqkv * ln_in_w[:, None]
    w13_f = _interleave_cols(res_w13 * ln_res_w[:, None], H)
    x_norm_post = x_norm * ln_post_w
    wsT = ws.transpose(0, 2, 1)  # [E, H, 2I]
    wsT_il = np.stack([_interleave_cols(wsT[e], I) for e in range(E)])
    w2sT = w2s.transpose(0, 2, 1)

    shared = {
        "tri": tri, "ident": bf(ident),
        "wqkv": bf(wqkv_f), "wo": bf(wo), "w13": bf(w13_f), "w2": bf(res_w2),
    }

    in_maps = []
    own = [[i, NBLK - 1 - i] for i in range(NC_)]
    for i in range(NC_):
        toks = np.concatenate([np.arange(b * BLK, (b + 1) * BLK) for b in own[i]])
        xT = hidden[toks].T
        cs = (cos_t[toks] * s[toks, None]).T
        sn = (sin_t[toks] * s[toks, None]).T
        scol = np.stack([s[toks[:BLK]], s[toks[BLK:]]], axis=1)
        bias = np.zeros((2, NBLK, BLK), np.float32)
        b0, b1 = own[i]
        bias[0, b0:, :] = NEG
        bias[1, b1:, :] = NEG
        exps = [2 * i, 2 * i + 1]
        xg = np.zeros((EPC, H, cap), np.float32)
        ew = np.zeros((EPC, BLK, cap), np.float32)
        for j, e in enumerate(exps):
            n = len(tok_lists[e])
            xg[j, :, :n] = x_norm_post[tok_lists[e]].T
            ew[j, :, :n] = wts[e][None, :]
        in_maps.append({
            "xT_bf": bf(xT), "xT32": f32(xT),
            "cos_s": f32(cs), "sin_s": f32(sn), "s_col": f32(scol),
            "bias": bias,
            "wsT": bf(wsT_il[exps]),
            "w2sT": bf(w2sT[exps]),
            "xgT": bf(xg), "ew": ew,
            **shared,
        })

    res = run_bass_kernel_spmd(nc, in_maps, core_ids=list(range(NC_)), trace=TRACE)
    LAST_RESULT = res

    out = np.zeros((T, H), np.float32)
    for i in range(NC_):
        toks = np.concatenate([np.arange(b * BLK, (b + 1) * BLK) for b in own[i]])
        out[toks] = res.results[i]["res_out"].T
    for i in range(NC_):
        for j, e in enumerate((2 * i, 2 * i + 1)):
            tl = tok_lists[e]
            out[tl] += res.results[i]["moe_out"][j].T[:len(tl)]
    return out


# revision 15
# speedup vs baseline: 1.6668x; 1.6668x over previous
"""ArcticDecoderLayer on 8 TRN2 NeuronCores.

Sharding:
  - tokens: zigzag block-parallel (core i owns 128-token blocks {i, 15-i});
    attention, wo, residual MLP are token-parallel (weights replicated).
  - MoE: expert-parallel, 2 experts/core; routing/top-2 + token gather/scatter
    done host-side (part of shard/unshard), expert GEMMs on device.
  - One AllGather (K^T feature-major + V token-major, bf16) is the only
    collective; causal masking is data-driven (per-core exp-bias columns) so
    the SPMD graph is identical on all cores.

All matmuls in bf16 (fp32 PE matmul is 4x slower); accumulation in f32 PSUM.
Weight streaming uses few large multi-k-chunk DMAs (3D APs) split across the
two HWDGE rings (sync + scalar) to keep sequencer issue cost off the critical
path.
"""
import numpy as np
import ml_dtypes

import concourse.bacc as bacc
import concourse.tile as tile
import concourse.mybir as mybir
from concourse.bass_utils import run_bass_kernel_spmd

F32 = mybir.dt.float32
BF16 = mybir.dt.bfloat16
AF = mybir.ActivationFunctionType

H = 2048
NH = 16
NKV = 4
HD = 128
HALF = 64
I = 1024
E = 16
TOPK = 2
T = 2048
EPS = 1e-5
THETA = 10000.0
NC_ = 8
BLK = 128
NBLK = 16
TPC = 256  # tokens per core
EPC = 2  # experts per core
SCALE = HD ** -0.5
NEG = -30000.0
KQ = H // BLK  # 16
MI = I // BLK  # 8

TRACE = False
DEBUG_TAPS = False
LAST_RESULT = None
_CACHE = {}

bf = lambda a: np.ascontiguousarray(np.asarray(a).astype(ml_dtypes.bfloat16))
f32 = lambda a: np.ascontiguousarray(a, dtype=np.float32)


def _slot(kb):
    c = min(kb, NBLK - 1 - kb)
    return 2 * c + (0 if kb < NC_ else 1)


def _build(cap):
    nc = bacc.Bacc("TRN2", target_bir_lowering=False, debug=False, num_devices=NC_)

    din = lambda name, shape, dt=BF16: nc.dram_tensor(name, shape, dt, kind="ExternalInput")
    xT_bf_d = din("xT_bf", [H, TPC])
    xT32_d = din("xT32", [H, TPC], F32)
    cos_d = din("cos_s", [HALF, TPC], F32)
    sin_d = din("sin_s", [HALF, TPC], F32)
    scol_d = din("s_col", [BLK, 2], F32)
    bias_d = din("bias", [2, NBLK, BLK], F32)
    tri_d = din("tri", [BLK, BLK], F32)
    ident_d = din("ident", [BLK, BLK])
    wqkv_d = din("wqkv", [H, NH * HD + 2 * NKV * HD])
    wo_d = din("wo", [NH * HD, H])
    w13_d = din("w13", [H, 2 * H])  # host-interleaved: chunk 2p=g_p, 2p+1=u_p
    w2_d = din("w2", [H, H])
    wsT_d = din("wsT", [EPC, H, 2 * I])  # host-interleaved g/u pairs
    w2sT_d = din("w2sT", [EPC, I, H])
    xg_d = din("xgT", [EPC, H, cap])
    ew_d = din("ew", [EPC, BLK, cap], F32)

    res_out_d = nc.dram_tensor("res_out", [H, TPC], F32, kind="ExternalOutput")
    moe_out_d = nc.dram_tensor("moe_out", [EPC, H, cap], F32, kind="ExternalOutput")
    taps = {}
    if DEBUG_TAPS:
        for nm, w in [("q", NH * TPC), ("k", NKV * TPC), ("v", 2 * NKV * HD),
                      ("kag", NBLK * NKV * BLK), ("vag", NBLK * NKV * BLK),
                      ("attnT", NH * TPC), ("resid", KQ * TPC), ("h2t", KQ * TPC),
                      ("gut", KQ * TPC)]:
            taps[nm] = nc.dram_tensor("tap_" + nm, [BLK, w],
                                      F32 if nm == "resid" else BF16,
                                      kind="ExternalOutput")

    KG = 4  # contraction chunks per weight-stream DMA
    SW = 8  # m-chunks per sweep for TN=256 GEMMs (4 paired psum banks)

    with tile.TileContext(nc) as tc:
        with (
            tc.tile_pool(name="res", bufs=1) as res,
            tc.tile_pool(name="stream", bufs=2) as stream,
            tc.tile_pool(name="small", bufs=3) as small,
            tc.tile_pool(name="outp", bufs=3) as outp,
            tc.tile_pool(name="acc", bufs=4, space="PSUM") as acc,
            tc.tile_pool(name="macc", bufs=4, space="PSUM") as macc,
            tc.tile_pool(name="dram", bufs=1, space="DRAM") as dram,
        ):
            eng_rr = [nc.sync, nc.scalar]

            # ---------------- resident loads ----------------
            xT_sb = res.tile([BLK, KQ * TPC], BF16, tag="xT")
            xT32_sb = res.tile([BLK, KQ * TPC], F32, tag="xT32")
            nc.sync.dma_start(xT_sb[:].rearrange("p (k t) -> p k t", k=KQ),
                              xT_bf_d.ap().rearrange("(k p) t -> p k t", p=BLK))
            nc.scalar.dma_start(xT32_sb[:].rearrange("p (k t) -> p k t", k=KQ),
                                xT32_d.ap().rearrange("(k p) t -> p k t", p=BLK))
            cos_sb = res.tile([HALF, TPC], F32, tag="cos")
            sin_sb = res.tile([HALF, TPC], F32, tag="sin")
            nc.sync.dma_start(cos_sb[:], cos_d[:])
            nc.sync.dma_start(sin_sb[:], sin_d[:])
            scol_sb = res.tile([BLK, 2], F32, tag="scol")
            nc.sync.dma_start(scol_sb[:], scol_d[:])
            bias_sb = res.tile([BLK, 2 * NBLK], F32, tag="bias")
            nc.sync.dma_start(bias_sb[:], bias_d.ap().rearrange("a k p -> p (a k)"))
            tri_sb = res.tile([BLK, BLK], F32, tag="tri")
            nc.sync.dma_start(tri_sb[:], tri_d[:])
            ident_sb = res.tile([BLK, BLK], BF16, tag="ident")
            nc.sync.dma_start(ident_sb[:], ident_d[:])
            ew_sb = res.tile([BLK, EPC * cap], F32, tag="ew")
            for e in range(EPC):
                nc.scalar.dma_start(ew_sb[:, e * cap:(e + 1) * cap], ew_d[e])
            xg_sb = res.tile([BLK, EPC * KQ * cap], BF16, tag="xg")
            for e in range(EPC):
                nc.scalar.dma_start(
                    xg_sb[:, e * KQ * cap:(e + 1) * KQ * cap].rearrange("p (k t) -> p k t", k=KQ),
                    xg_d[e].rearrange("(k p) t -> p k t", p=BLK))

            ones_bf = res.tile([BLK, 1], BF16, tag="onesb")
            nc.vector.memset(ones_bf[:], 1.0)
            ones_row = res.tile([1, BLK], F32, tag="onesr")
            nc.vector.memset(ones_row[:], 1.0)
            ones_cf = res.tile([BLK, 1], F32, tag="onescf")
            nc.vector.memset(ones_cf[:], 1.0)

            q_sb = res.tile([BLK, NH * TPC], BF16, tag="q")
            k_sb = res.tile([BLK, NKV * TPC], BF16, tag="k")
            v_sb = res.tile([BLK, 2 * NKV * HD], BF16, tag="v")
            attnT_sb = res.tile([BLK, NH * TPC], BF16, tag="attnT")
            resid_sb = res.tile([BLK, KQ * TPC], F32, tag="resid")
            h2_sb = res.tile([BLK, KQ * TPC], BF16, tag="h2")
            gu_sb = res.tile([BLK, KQ * TPC], BF16, tag="gu")
            hm_sb = res.tile([BLK, EPC * MI * cap], BF16, tag="hm")
            kag_sb = res.tile([BLK, NBLK * NKV * BLK], BF16, tag="kag")
            vag_sb = res.tile([BLK, NBLK * NKV * BLK], BF16, tag="vag")

            # ============ generic streamed GEMM sweep ============
            # out chunk m = w[:, m*128:(m+1)*128].T @ rhs ; accumulate over kcnt
            # contraction chunks. consume(m, ps_ap) or consume_pair(j, ps, ms).
            def gemm(w_src, mcnt, kcnt, rhs_fn, pool, tn, consume, sweep, dma_eng, tag):
                pair = 2 * tn <= 512
                for s0 in range(0, mcnt, sweep):
                    ms = list(range(s0, min(s0 + sweep, mcnt)))
                    mw = len(ms)
                    if pair:
                        nt = (mw + 1) // 2
                        pts = [pool.tile([BLK, 2 * tn], F32, tag=tag,
                                         name=f"pt{j}") for j in range(nt)]
                        paps = [pts[j // 2][:, (j % 2) * tn:(j % 2 + 1) * tn]
                                for j in range(mw)]
                    else:
                        pts = [pool.tile([BLK, tn], F32, tag=tag,
                                         name=f"pt{j}") for j in range(mw)]
                        paps = [pts[j][:] for j in range(mw)]
                    for kg0 in range(0, kcnt, KG):
                        kgn = min(KG, kcnt - kg0)
                        wt = stream.tile([BLK, KG * SW * BLK], BF16, tag="wt")
                        dma_eng[(kg0 // KG) % 2].dma_start(
                            wt[:, :kgn * mw * BLK].rearrange("p (k c) -> p k c", k=kgn),
                            w_src(kg0, kgn, ms[0] * BLK, (ms[-1] + 1) * BLK))
                        for kl in range(kgn):
                            k = kg0 + kl
                            for j in range(mw):
                                # start=True clears the WHOLE psum bank, so for
                                # paired chunks only the first chunk of a tile
                                # may set it (partner then overwrites via
                                # cleared has_written bits).
                                first = (j % 2 == 0) if pair else True
                                last = (j % 2 == 1 or j == mw - 1) if pair else True
                                nc.tensor.matmul(
                                    paps[j], wt[:, (kl * mw + j) * BLK:(kl * mw + j + 1) * BLK],
                                    rhs_fn(k), start=(k == 0 and first),
                                    stop=(k == kcnt - 1 and last))
                    consume(ms, paps, pts)

            # ---------------- QKV projection (feature-major out) ----------------
            def rope_chunk(ps, dst, col0):
                t1 = small.tile([HALF, TPC], F32, tag="r1")
                t2 = small.tile([HALF, TPC], F32, tag="r2")
                nc.vector.tensor_mul(t1[:], ps[0:HALF, :], cos_sb[:])
                nc.vector.tensor_mul(t2[:], ps[HALF:BLK, :], sin_sb[:])
                nc.vector.tensor_sub(dst[0:HALF, col0:col0 + TPC], t1[:], t2[:])
                t3 = small.tile([HALF, TPC], F32, tag="r1")
                t4 = small.tile([HALF, TPC], F32, tag="r2")
                nc.vector.tensor_mul(t3[:], ps[HALF:BLK, :], cos_sb[:])
                nc.vector.tensor_mul(t4[:], ps[0:HALF, :], sin_sb[:])
                nc.vector.tensor_add(dst[HALF:BLK, col0:col0 + TPC], t3[:], t4[:])

            def qkv_consume(ms, paps, pts):
                for m, ps in zip(ms, paps):
                    if m < NH:
                        rope_chunk(ps, q_sb, m * TPC)
                    elif m < NH + NKV:
                        rope_chunk(ps, k_sb, (m - NH) * TPC)
                    else:
                        kvh = m - NH - NKV
                        vtmp = small.tile([BLK, TPC], BF16, tag="vtmp")
                        nc.scalar.activation(vtmp[:], ps, AF.Copy)
                        for tb in range(2):
                            pt = macc.tile([BLK, BLK], BF16, tag="macct")
                            nc.tensor.transpose(pt[:], vtmp[:, tb * BLK:(tb + 1) * BLK], ident_sb[:])
                            nc.vector.tensor_scalar_mul(
                                v_sb[:, (tb * NKV + kvh) * BLK:(tb * NKV + kvh + 1) * BLK],
                                pt[:], scol_sb[:, tb:tb + 1])

            wqkv_src = lambda kg0, kgn, c0, c1: wqkv_d[kg0 * BLK:(kg0 + kgn) * BLK, c0:c1] \
                .rearrange("(k p) c -> p k c", p=BLK)
            gemm(wqkv_src, NH + 2 * NKV, KQ,
                 lambda k: xT_sb[:, k * TPC:(k + 1) * TPC], acc, TPC, qkv_consume,
                 SW, eng_rr, "acct")

            # ---------------- KV AllGather ----------------
            KSZ = NKV * BLK * TPC  # 131072 elems
            kv_local = dram.tile([2, KSZ], BF16)
            kv_ag = dram.tile([NC_, 2, KSZ], BF16, addr_space="Shared")
            nc.sync.dma_start(
                kv_local[0, :].rearrange("(h d t) -> d h t", h=NKV, d=BLK),
                k_sb[:].rearrange("d (h t) -> d h t", h=NKV))
            nc.sync.dma_start(
                kv_local[1, :].rearrange("(b h t d) -> t b h d", b=2, h=NKV, t=BLK),
                v_sb[:].rearrange("t (b h d) -> t b h d", b=2, h=NKV))
            nc.gpsimd.collective_compute(
                "AllGather", mybir.AluOpType.bypass,
                replica_groups=[list(range(NC_))],
                ins=[kv_local[:]], outs=[kv_ag[:]])
            # unpack in AG order (slot = 2c+sub); attention maps kb -> slot.
            for sub in range(2):
                for hh in range(NKV):
                    nc.sync.dma_start(
                        kag_sb[:].rearrange("d (c s h t) -> s h d c t", c=NC_, s=2, h=NKV)[sub, hh],
                        kv_ag[:, 0, :].rearrange("c (h d s t) -> s h d c t", h=NKV, d=BLK, s=2)[sub, hh])
                    nc.scalar.dma_start(
                        vag_sb[:].rearrange("t (c s h dd) -> s h t c dd", c=NC_, s=2, h=NKV)[sub, hh],
                        kv_ag[:, 1, :].rearrange("c (s h t dd) -> s h t c dd", s=2, h=NKV, t=BLK)[sub, hh])

            # ---------------- MoE (overlaps AG + attention) ----------------
            for e in range(EPC):
                rhs_e = lambda k, e=e: xg_sb[:, (e * KQ + k) * cap:(e * KQ + k + 1) * cap]
                pend_g = {}

                def gu_consume(ms, paps, pts, e=e, pend_g=pend_g):
                    for m, ps, pt in zip(ms, paps, pts):
                        if m % 2 == 0:
                            pend_g[m] = (ps, pt)
                        else:
                            gps, _ = pend_g.pop(m - 1)
                            sg = small.tile([BLK, cap], BF16, tag="sg")
                            nc.scalar.activation(sg[:], gps, AF.Silu)
                            p = m // 2
                            nc.vector.tensor_mul(
                                hm_sb[:, (e * MI + p) * cap:(e * MI + p + 1) * cap],
                                sg[:], ps)

                ws_src = lambda kg0, kgn, c0, c1, e=e: wsT_d[e, kg0 * BLK:(kg0 + kgn) * BLK, c0:c1] \
                    .rearrange("(k p) c -> p k c", p=BLK)
                gemm(ws_src, 2 * MI, KQ, rhs_e, macc, cap, gu_consume, 4, eng_rr, "macct")

                def w2s_consume(ms, paps, pts, e=e):
                    for m, ps in zip(ms, paps):
                        mo = outp.tile([BLK, cap], F32, tag="mo")
                        nc.vector.tensor_mul(mo[:], ps, ew_sb[:, e * cap:(e + 1) * cap])
                        nc.scalar.dma_start(moe_out_d[e, m * BLK:(m + 1) * BLK, :], mo[:])

                w2s_src = lambda kg0, kgn, c0, c1, e=e: w2sT_d[e, kg0 * BLK:(kg0 + kgn) * BLK, c0:c1] \
                    .rearrange("(k p) c -> p k c", p=BLK)
                gemm(w2s_src, KQ, MI,
                     lambda k, e=e: hm_sb[:, (e * MI + k) * cap:(e * MI + k + 1) * cap],
                     macc, cap, w2s_consume, 4, eng_rr, "macct")

            # ---------------- attention ----------------
            for h in range(NH):
                kvh = h // (NH // NKV)
                for qb in range(2):
                    nkb = 8 if qb == 0 else 16
                    qv = q_sb[:, h * TPC + qb * BLK: h * TPC + (qb + 1) * BLK]
                    aps = acc.tile([BLK, BLK], F32, tag="acct")
                    dps = acc.tile([1, BLK], F32, tag="acct")
                    # diagonal (own tokens, static tri mask)
                    sps = acc.tile([BLK, BLK], F32, tag="acct")
                    nc.tensor.matmul(
                        sps[:], k_sb[:, kvh * TPC + qb * BLK: kvh * TPC + (qb + 1) * BLK],
                        qv, start=True, stop=True)
                    stmp = small.tile([BLK, BLK], F32, tag="stmp")
                    nc.vector.tensor_add(stmp[:], sps[:], tri_sb[:])
                    pd = small.tile([BLK, BLK], BF16, tag="pd")
                    nc.scalar.activation(pd[:], stmp[:], AF.Exp, scale=SCALE)
                    nc.tensor.matmul(aps[:], v_sb[:, (qb * NKV + kvh) * BLK:(qb * NKV + kvh + 1) * BLK],
                                     pd[:], start=True, stop=False)
                    nc.tensor.matmul(dps[:], ones_bf[:], pd[:], start=True, stop=False)
                    # dense blocks (AG buffer, slot order), mask via bias column
                    for kb in range(nkb):
                        sl = _slot(kb)
                        sps2 = acc.tile([BLK, BLK], F32, tag="acct")
                        nc.tensor.matmul(
                            sps2[:], kag_sb[:, (sl * NKV + kvh) * BLK:(sl * NKV + kvh + 1) * BLK],
                            qv, start=True, stop=True)
                        pdn = small.tile([BLK, BLK], BF16, tag="pd")
                        nc.scalar.activation(
                            pdn[:], sps2[:], AF.Exp, scale=SCALE,
                            bias=bias_sb[:, qb * NBLK + kb: qb * NBLK + kb + 1])
                        last = kb == nkb - 1
                        nc.tensor.matmul(aps[:], vag_sb[:, (sl * NKV + kvh) * BLK:(sl * NKV + kvh + 1) * BLK],
                                         pdn[:], start=False, stop=last)
                        nc.tensor.matmul(dps[:], ones_bf[:], pdn[:], start=False, stop=last)
                    # normalize: recip(denom) -> K=1 broadcast matmul -> mul
                    rec = small.tile([1, BLK], F32, tag="rec")
                    nc.vector.reciprocal(rec[:], dps[:])
                    bcp = acc.tile([BLK, BLK], F32, tag="acct")
                    nc.tensor.matmul(bcp[:], ones_row[:], rec[:], start=True, stop=True)
                    bcs = small.tile([BLK, BLK], F32, tag="bcs")
                    nc.scalar.activation(bcs[:], bcp[:], AF.Copy)
                    nc.vector.tensor_mul(
                        attnT_sb[:, h * TPC + qb * BLK: h * TPC + (qb + 1) * BLK],
                        aps[:], bcs[:])

            # ---------------- wo + residual ----------------
            def wo_consume(ms, paps, pts):
                for jt, pt in enumerate(pts):
                    m0 = ms[2 * jt]
                    w = pt.shape[1]
                    nc.vector.tensor_add(resid_sb[:, m0 * TPC: m0 * TPC + w],
                                         pt[:], xT32_sb[:, m0 * TPC: m0 * TPC + w])

            wo_src = lambda kg0, kgn, c0, c1: wo_d[kg0 * BLK:(kg0 + kgn) * BLK, c0:c1] \
                .rearrange("(k p) c -> p k c", p=BLK)
            gemm(wo_src, KQ, KQ, lambda k: attnT_sb[:, k * TPC:(k + 1) * TPC],
                 acc, TPC, wo_consume, SW, eng_rr, "acct")

            # ---------------- residual MLP norm scale ----------------
            ssq = acc.tile([1, TPC], F32, tag="acct")
            for k in range(KQ):
                sq = small.tile([BLK, TPC], F32, tag="sq")
                nc.vector.tensor_mul(sq[:], resid_sb[:, k * TPC:(k + 1) * TPC],
                                     resid_sb[:, k * TPC:(k + 1) * TPC])
                nc.tensor.matmul(ssq[:], ones_cf[:], sq[:],
                                 start=(k == 0), stop=(k == KQ - 1))
            vtmp2 = small.tile([1, TPC], F32, tag="vt")
            nc.vector.tensor_scalar(vtmp2[:], ssq[:], 1.0 / H, EPS,
                                    mybir.AluOpType.mult, mybir.AluOpType.add)
            st = small.tile([1, TPC], F32, tag="vt2")
            nc.scalar.activation(st[:], vtmp2[:], AF.Sqrt)
            s2r = small.tile([1, TPC], F32, tag="vt3")
            nc.vector.reciprocal(s2r[:], st[:])
            s2p = acc.tile([BLK, TPC], F32, tag="acct")
            nc.tensor.matmul(s2p[:], ones_row[:], s2r[:], start=True, stop=True)
            s2s = small.tile([BLK, TPC], F32, tag="s2s")
            nc.scalar.activation(s2s[:], s2p[:], AF.Copy)
            for k in range(KQ):
                nc.vector.tensor_mul(h2_sb[:, k * TPC:(k + 1) * TPC],
                                     resid_sb[:, k * TPC:(k + 1) * TPC], s2s[:])

            # ---------------- w13 (interleaved g/u) + silu_and_mul ----------------
            def w13_consume(ms, paps, pts):
                for jt, pt in enumerate(pts):
                    p = ms[2 * jt] // 2
                    sg = small.tile([BLK, TPC], BF16, tag="sg13")
                    nc.scalar.activation(sg[:], pt[:, 0:TPC], AF.Silu)
                    nc.vector.tensor_mul(gu_sb[:, p * TPC:(p + 1) * TPC],
                                         sg[:], pt[:, TPC:2 * TPC])

            w13_src = lambda kg0, kgn, c0, c1: w13_d[kg0 * BLK:(kg0 + kgn) * BLK, c0:c1] \
                .rearrange("(k p) c -> p k c", p=BLK)
            gemm(w13_src, 2 * KQ, KQ, lambda k: h2_sb[:, k * TPC:(k + 1) * TPC],
                 acc, TPC, w13_consume, SW, eng_rr, "acct")

            # ---------------- w2 + final out ----------------
            def w2_consume(ms, paps, pts):
                for jt, pt in enumerate(pts):
                    m0 = ms[2 * jt]
                    w = pt.shape[1]
                    fo = outp.tile([BLK, 2 * TPC], F32, tag="fo")
                    nc.vector.tensor_add(fo[:, :w], pt[:], resid_sb[:, m0 * TPC:m0 * TPC + w])
                    nc.sync.dma_start(
                        res_out_d.ap().rearrange("(m p) t -> p m t", p=BLK)[:, m0:m0 + w // TPC],
                        fo[:, :w].rearrange("p (m t) -> p m t", t=TPC))

            w2_src = lambda kg0, kgn, c0, c1: w2_d[kg0 * BLK:(kg0 + kgn) * BLK, c0:c1] \
                .rearrange("(k p) c -> p k c", p=BLK)
            gemm(w2_src, KQ, KQ, lambda k: gu_sb[:, k * TPC:(k + 1) * TPC],
                 acc, TPC, w2_consume, SW, eng_rr, "acct")

            if DEBUG_TAPS:
                for nm, sb in [("q", q_sb), ("k", k_sb), ("v", v_sb),
                               ("kag", kag_sb), ("vag", vag_sb), ("attnT", attnT_sb),
                               ("resid", resid_sb), ("h2t", h2_sb), ("gut", gu_sb)]:
                    nc.sync.dma_start(taps[nm].ap(), sb[:])

    nc.compile()
    return nc


def _interleave_cols(w, half):
    # [rows, 2*half] -> column chunks reordered so chunk 2p=g_p, 2p+1=u_p
    rows = w.shape[0]
    g = w[:, :half].reshape(rows, half // BLK, BLK)
    u = w[:, half:].reshape(rows, half // BLK, BLK)
    out = np.empty((rows, 2 * (half // BLK), BLK), w.dtype)
    out[:, 0::2] = g
    out[:, 1::2] = u
    return out.reshape(rows, 2 * half // BLK * BLK)


def kernel(**inputs):
    global LAST_RESULT
    hidden = f32(inputs["hidden_states"])
    positions = np.asarray(inputs["positions"]).astype(np.float32)
    ln_in_w = f32(inputs["ln_in_w"])
    ln_post_w = f32(inputs["ln_post_w"])
    ln_res_w = f32(inputs["ln_res_w"])
    wqkv = f32(inputs["wqkv"])
    wo = f32(inputs["wo"])
    res_w13 = f32(inputs["res_w13"])
    res_w2 = f32(inputs["res_w2"])
    gate_w = f32(inputs["gate_w"])
    ws = f32(inputs["ws"])
    w2s = f32(inputs["w2s"])

    # ---- host prep (sharding) ----
    s = 1.0 / np.sqrt(np.mean(hidden * hidden, axis=1) + EPS)  # [T]
    x_norm = hidden * s[:, None]

    logits = (x_norm * ln_post_w) @ gate_w
    pr = np.exp(logits - logits.max(-1, keepdims=True))
    pr /= pr.sum(-1, keepdims=True)
    topi = np.argsort(-pr, axis=-1, kind="stable")[:, :TOPK]
    topw = np.take_along_axis(pr, topi, axis=-1)
    topw /= topw.sum(-1, keepdims=True)
    tok_lists = [np.where((topi == e).any(-1))[0] for e in range(E)]
    wts = [np.sum(np.where(topi[tl] == e, topw[tl], 0.0), -1).astype(np.float32)
           for e, tl in zip(range(E), tok_lists)]
    cap = max(128, -(-max(len(t) for t in tok_lists) // 64) * 64)
    assert cap <= 512, cap

    ck = (cap, DEBUG_TAPS)
    if ck not in _CACHE:
        _CACHE[ck] = _build(cap)
    nc = _CACHE[ck]

    inv_freq = 1.0 / (THETA ** (np.arange(0, HD, 2, dtype=np.float32) / HD))
    ang = positions[:, None] * inv_freq
    cos_t, sin_t = np.cos(ang), np.sin(ang)

    tri = np.where(np.arange(BLK)[None, :] >= np.arange(BLK)[:, None], 0.0, NEG).astype(np.float32)
    ident = np.eye(BLK, dtype=np.float32)

    wqkv_f = wqkv * ln_in_w[:, None]
    w13_f = _interleave_cols(res_w13 * ln_res_w[:, None], H)
    x_norm_post = x_norm * ln_post_w
    wsT = ws.transpose(0, 2, 1)  # [E, H, 2I]
    wsT_il = np.stack([_interleave_cols(wsT[e], I) for e in range(E)])
    w2sT = w2s.transpose(0, 2, 1)

    shared = {
        "tri": tri, "ident": bf(ident),
        "wqkv": bf(wqkv_f), "wo": bf(wo), "w13": bf(w13_f), "w2": bf(res_w2),
    }

    in_maps = []
    own = [[i, NBLK - 1 - i] for i in range(NC_)]
    for i in range(NC_):
        toks = np.concatenate([np.arange(b * BLK, (b + 1) * BLK) for b in own[i]])
        xT = hidden[toks].T
        cs = (cos_t[toks] * s[toks, None]).T
        sn = (sin_t[toks] * s[toks, None]).T
        scol = np.stack([s[toks[:BLK]], s[toks[BLK:]]], axis=1)
        bias = np.zeros((2, NBLK, BLK), np.float32)
        b0, b1 = own[i]
        bias[0, b0:, :] = NEG
        bias[1, b1:, :] = NEG
        exps = [2 * i, 2 * i + 1]
        xg = np.zeros((EPC, H, cap), np.float32)
        ew = np.zeros((EPC, BLK, cap), np.float32)
        for j, e in enumerate(exps):
            n = len(tok_lists[e])
            xg[j, :, :n] = x_norm_post[tok_lists[e]].T
            ew[j, :, :n] = wts[e][None, :]
        in_maps.append({
            "xT_bf": bf(xT), "xT32": f32(xT),
            "cos_s": f32(cs), "sin_s": f32(sn), "s_col": f32(scol),
            "bias": bias,
            "wsT": bf(wsT_il[exps]),
            "w2sT": bf(w2sT[exps]),
            "xgT": bf(xg), "ew": ew,
            **shared,
        })

    res = run_bass_kernel_spmd(nc, in_maps, core_ids=list(range(NC_)), trace=TRACE)
    LAST_RESULT = res

    out = np.zeros((T, H), np.float32)
    for i in range(NC_):
        toks = np.concatenate([np.arange(b * BLK, (b + 1) * BLK) for b in own[i]])
        out[toks] = res.results[i]["res_out"].T
    for i in range(NC_):
        for j, e in enumerate((2 * i, 2 * i + 1)):
            tl = tok_lists[e]
            out[tl] += res.results[i]["moe_out"][j].T[:len(tl)]
    return out


# revision 16
# speedup vs baseline: 1.8840x; 1.1303x over previous
"""ArcticDecoderLayer on 8 TRN2 NeuronCores.

Sharding:
  - tokens: zigzag block-parallel (core i owns 128-token blocks {i, 15-i});
    attention, wo, residual MLP are token-parallel (weights replicated).
  - MoE: expert-parallel, 2 experts/core; routing/top-2 + token gather/scatter
    done host-side (part of shard/unshard), expert GEMMs on device.
  - One AllGather (K^T feature-major + V token-major, bf16) is the only
    collective; causal masking is data-driven (per-core exp-bias columns) so
    the SPMD graph is identical on all cores.

All matmuls in bf16 (fp32 PE matmul is 4x slower); accumulation in f32 PSUM.
Weight streaming uses few large multi-k-chunk DMAs (3D APs) split across the
two HWDGE rings (sync + scalar) to keep sequencer issue cost off the critical
path.
"""
import numpy as np
import ml_dtypes

import concourse.bacc as bacc
import concourse.tile as tile
import concourse.mybir as mybir
from concourse.bass_utils import run_bass_kernel_spmd

F32 = mybir.dt.float32
BF16 = mybir.dt.bfloat16
AF = mybir.ActivationFunctionType

H = 2048
NH = 16
NKV = 4
HD = 128
HALF = 64
I = 1024
E = 16
TOPK = 2
T = 2048
EPS = 1e-5
THETA = 10000.0
NC_ = 8
BLK = 128
NBLK = 16
TPC = 256  # tokens per core
EPC = 2  # experts per core
SCALE = HD ** -0.5
NEG = -30000.0
KQ = H // BLK  # 16
MI = I // BLK  # 8

TRACE = False
DEBUG_TAPS = False
LAST_RESULT = None
_CACHE = {}

bf = lambda a: np.ascontiguousarray(np.asarray(a).astype(ml_dtypes.bfloat16))
f32 = lambda a: np.ascontiguousarray(a, dtype=np.float32)


def _slot(kb):
    c = min(kb, NBLK - 1 - kb)
    return 2 * c + (0 if kb < NC_ else 1)


def _build(cap):
    nc = bacc.Bacc("TRN2", target_bir_lowering=False, debug=False, num_devices=NC_)

    din = lambda name, shape, dt=BF16: nc.dram_tensor(name, shape, dt, kind="ExternalInput")
    xT_bf_d = din("xT_bf", [H, TPC])
    xT32_d = din("xT32", [H, TPC], F32)
    cos_d = din("cos_s", [HALF, TPC], F32)
    sin_d = din("sin_s", [HALF, TPC], F32)
    scol_d = din("s_col", [BLK, 2], F32)
    bias_d = din("bias", [2, NBLK, BLK], F32)
    tri_d = din("tri", [BLK, BLK], F32)
    ident_d = din("ident", [BLK, BLK])
    wqkv_d = din("wqkv", [H, NH * HD + 2 * NKV * HD])
    wo_d = din("wo", [NH * HD, H])
    w13_d = din("w13", [H, 2 * H])  # host-interleaved: chunk 2p=g_p, 2p+1=u_p
    w2_d = din("w2", [H, H])
    wsT_d = din("wsT", [EPC, H, 2 * I])  # host-interleaved g/u pairs
    w2sT_d = din("w2sT", [EPC, I, H])
    xg_d = din("xgT", [EPC, H, cap])
    ew_d = din("ew", [EPC, BLK, cap], F32)

    res_out_d = nc.dram_tensor("res_out", [H, TPC], F32, kind="ExternalOutput")
    moe_out_d = nc.dram_tensor("moe_out", [EPC, H, cap], F32, kind="ExternalOutput")
    taps = {}
    if DEBUG_TAPS:
        for nm, w in [("q", NH * TPC), ("k", NKV * TPC), ("v", 2 * NKV * HD),
                      ("kag", NBLK * NKV * BLK), ("vag", NBLK * NKV * BLK),
                      ("attnT", NH * TPC), ("resid", KQ * TPC), ("h2t", KQ * TPC),
                      ("gut", KQ * TPC)]:
            taps[nm] = nc.dram_tensor("tap_" + nm, [BLK, w],
                                      F32 if nm == "resid" else BF16,
                                      kind="ExternalOutput")

    KG = 4  # contraction chunks per weight-stream DMA
    SW = 8  # m-chunks per sweep for TN=256 GEMMs (4 paired psum banks)

    with tile.TileContext(nc) as tc:
        with (
            tc.tile_pool(name="res", bufs=1) as res,
            tc.tile_pool(name="stream", bufs=2) as stream,
            tc.tile_pool(name="small", bufs=3) as small,
            tc.tile_pool(name="outp", bufs=3) as outp,
            tc.tile_pool(name="acc", bufs=4, space="PSUM") as acc,
            tc.tile_pool(name="macc", bufs=4, space="PSUM") as macc,
            tc.tile_pool(name="dram", bufs=1, space="DRAM") as dram,
        ):
            eng_rr = [nc.sync, nc.scalar]

            # ---------------- resident loads ----------------
            xT_sb = res.tile([BLK, KQ * TPC], BF16, tag="xT")
            xT32_sb = res.tile([BLK, KQ * TPC], F32, tag="xT32")
            nc.sync.dma_start(xT_sb[:].rearrange("p (k t) -> p k t", k=KQ),
                              xT_bf_d.ap().rearrange("(k p) t -> p k t", p=BLK))
            nc.scalar.dma_start(xT32_sb[:].rearrange("p (k t) -> p k t", k=KQ),
                                xT32_d.ap().rearrange("(k p) t -> p k t", p=BLK))
            cos_sb = res.tile([HALF, TPC], F32, tag="cos")
            sin_sb = res.tile([HALF, TPC], F32, tag="sin")
            nc.sync.dma_start(cos_sb[:], cos_d[:])
            nc.sync.dma_start(sin_sb[:], sin_d[:])
            scol_sb = res.tile([BLK, 2], F32, tag="scol")
            nc.sync.dma_start(scol_sb[:], scol_d[:])
            bias_sb = res.tile([BLK, 2 * NBLK], F32, tag="bias")
            nc.sync.dma_start(bias_sb[:], bias_d.ap().rearrange("a k p -> p (a k)"))
            tri_sb = res.tile([BLK, BLK], F32, tag="tri")
            nc.sync.dma_start(tri_sb[:], tri_d[:])
            ident_sb = res.tile([BLK, BLK], BF16, tag="ident")
            nc.sync.dma_start(ident_sb[:], ident_d[:])
            ew_sb = res.tile([BLK, EPC * cap], F32, tag="ew")
            for e in range(EPC):
                nc.scalar.dma_start(ew_sb[:, e * cap:(e + 1) * cap], ew_d[e])
            xg_sb = res.tile([BLK, EPC * KQ * cap], BF16, tag="xg")
            for e in range(EPC):
                nc.scalar.dma_start(
                    xg_sb[:, e * KQ * cap:(e + 1) * KQ * cap].rearrange("p (k t) -> p k t", k=KQ),
                    xg_d[e].rearrange("(k p) t -> p k t", p=BLK))

            ones_bf = res.tile([BLK, 1], BF16, tag="onesb")
            nc.vector.memset(ones_bf[:], 1.0)
            ones_row = res.tile([1, BLK], F32, tag="onesr")
            nc.vector.memset(ones_row[:], 1.0)
            ones_cf = res.tile([BLK, 1], F32, tag="onescf")
            nc.vector.memset(ones_cf[:], 1.0)

            q_sb = res.tile([BLK, NH * TPC], BF16, tag="q")
            k_sb = res.tile([BLK, NKV * TPC], BF16, tag="k")
            v_sb = res.tile([BLK, 2 * NKV * HD], BF16, tag="v")
            attnT_sb = res.tile([BLK, NH * TPC], BF16, tag="attnT")
            resid_sb = res.tile([BLK, KQ * TPC], F32, tag="resid")
            h2_sb = res.tile([BLK, KQ * TPC], BF16, tag="h2")
            gu_sb = res.tile([BLK, KQ * TPC], BF16, tag="gu")
            hm_sb = res.tile([BLK, EPC * MI * cap], BF16, tag="hm")
            kag_sb = res.tile([BLK, NBLK * NKV * BLK], BF16, tag="kag")
            vag_sb = res.tile([BLK, NBLK * NKV * BLK], BF16, tag="vag")

            # ============ generic streamed GEMM sweep ============
            # out chunk m = w[:, m*128:(m+1)*128].T @ rhs ; accumulate over kcnt
            # contraction chunks. consume(m, ps_ap) or consume_pair(j, ps, ms).
            def gemm(w_src, mcnt, kcnt, rhs_fn, pool, tn, consume, sweep, dma_eng, tag,
                     sweep_starts=None):
                pair = 2 * tn <= 512
                for s0 in (sweep_starts if sweep_starts is not None
                           else range(0, mcnt, sweep)):
                    ms = list(range(s0, min(s0 + sweep, mcnt)))
                    mw = len(ms)
                    if pair:
                        nt = (mw + 1) // 2
                        pts = [pool.tile([BLK, 2 * tn], F32, tag=tag,
                                         name=f"pt{j}") for j in range(nt)]
                        paps = [pts[j // 2][:, (j % 2) * tn:(j % 2 + 1) * tn]
                                for j in range(mw)]
                    else:
                        pts = [pool.tile([BLK, tn], F32, tag=tag,
                                         name=f"pt{j}") for j in range(mw)]
                        paps = [pts[j][:] for j in range(mw)]
                    for kg0 in range(0, kcnt, KG):
                        kgn = min(KG, kcnt - kg0)
                        wt = stream.tile([BLK, KG * SW * BLK], BF16, tag="wt")
                        dma_eng[(kg0 // KG) % 2].dma_start(
                            wt[:, :kgn * mw * BLK].rearrange("p (k c) -> p k c", k=kgn),
                            w_src(kg0, kgn, ms[0] * BLK, (ms[-1] + 1) * BLK))
                        for kl in range(kgn):
                            k = kg0 + kl
                            for j in range(mw):
                                # start=True clears the WHOLE psum bank, so for
                                # paired chunks only the first chunk of a tile
                                # may set it (partner then overwrites via
                                # cleared has_written bits).
                                first = (j % 2 == 0) if pair else True
                                last = (j % 2 == 1 or j == mw - 1) if pair else True
                                nc.tensor.matmul(
                                    paps[j], wt[:, (kl * mw + j) * BLK:(kl * mw + j + 1) * BLK],
                                    rhs_fn(k), start=(k == 0 and first),
                                    stop=(k == kcnt - 1 and last))
                    consume(ms, paps, pts)

            # ---------------- QKV projection (feature-major out) ----------------
            def rope_chunk(ps, dst, col0):
                t1 = small.tile([HALF, TPC], F32, tag="r1")
                t2 = small.tile([HALF, TPC], F32, tag="r2")
                nc.vector.tensor_mul(t1[:], ps[0:HALF, :], cos_sb[:])
                nc.vector.tensor_mul(t2[:], ps[HALF:BLK, :], sin_sb[:])
                nc.vector.tensor_sub(dst[0:HALF, col0:col0 + TPC], t1[:], t2[:])
                t3 = small.tile([HALF, TPC], F32, tag="r1")
                t4 = small.tile([HALF, TPC], F32, tag="r2")
                nc.vector.tensor_mul(t3[:], ps[HALF:BLK, :], cos_sb[:])
                nc.vector.tensor_mul(t4[:], ps[0:HALF, :], sin_sb[:])
                nc.vector.tensor_add(dst[HALF:BLK, col0:col0 + TPC], t3[:], t4[:])

            def qkv_consume(ms, paps, pts):
                for m, ps in zip(ms, paps):
                    if m < NH:
                        rope_chunk(ps, q_sb, m * TPC)
                    elif m < NH + NKV:
                        rope_chunk(ps, k_sb, (m - NH) * TPC)
                    else:
                        kvh = m - NH - NKV
                        vtmp = small.tile([BLK, TPC], BF16, tag="vtmp")
                        nc.scalar.activation(vtmp[:], ps, AF.Copy)
                        for tb in range(2):
                            pt = macc.tile([BLK, BLK], BF16, tag="macct")
                            nc.tensor.transpose(pt[:], vtmp[:, tb * BLK:(tb + 1) * BLK], ident_sb[:])
                            nc.vector.tensor_scalar_mul(
                                v_sb[:, (tb * NKV + kvh) * BLK:(tb * NKV + kvh + 1) * BLK],
                                pt[:], scol_sb[:, tb:tb + 1])

            wqkv_src = lambda kg0, kgn, c0, c1: wqkv_d[kg0 * BLK:(kg0 + kgn) * BLK, c0:c1] \
                .rearrange("(k p) c -> p k c", p=BLK)
            gemm(wqkv_src, NH + 2 * NKV, KQ,
                 lambda k: xT_sb[:, k * TPC:(k + 1) * TPC], acc, TPC, qkv_consume,
                 SW, eng_rr, "acct")

            # ---------------- KV AllGather ----------------
            KSZ = NKV * BLK * TPC  # 131072 elems
            kv_local = dram.tile([2, KSZ], BF16)
            kv_ag = dram.tile([NC_, 2, KSZ], BF16, addr_space="Shared")
            nc.sync.dma_start(
                kv_local[0, :].rearrange("(h d t) -> d h t", h=NKV, d=BLK),
                k_sb[:].rearrange("d (h t) -> d h t", h=NKV))
            nc.sync.dma_start(
                kv_local[1, :].rearrange("(b h t d) -> t b h d", b=2, h=NKV, t=BLK),
                v_sb[:].rearrange("t (b h d) -> t b h d", b=2, h=NKV))
            nc.gpsimd.collective_compute(
                "AllGather", mybir.AluOpType.bypass,
                replica_groups=[list(range(NC_))],
                ins=[kv_local[:]], outs=[kv_ag[:]])
            # unpack in AG order (slot = 2c+sub); attention maps kb -> slot.
            for sub in range(2):
                for hh in range(NKV):
                    nc.sync.dma_start(
                        kag_sb[:].rearrange("d (c s h t) -> s h d c t", c=NC_, s=2, h=NKV)[sub, hh],
                        kv_ag[:, 0, :].rearrange("c (h d s t) -> s h d c t", h=NKV, d=BLK, s=2)[sub, hh])
                    nc.scalar.dma_start(
                        vag_sb[:].rearrange("t (c s h dd) -> s h t c dd", c=NC_, s=2, h=NKV)[sub, hh],
                        kv_ag[:, 1, :].rearrange("c (s h t dd) -> s h t c dd", s=2, h=NKV, t=BLK)[sub, hh])

            # ------- MoE sweeps (as thunks) interleaved with attention heads -------
            moe_thunks = []
            for e in range(EPC):
                rhs_e = lambda k, e=e: xg_sb[:, (e * KQ + k) * cap:(e * KQ + k + 1) * cap]
                pend_g = {}

                def gu_consume(ms, paps, pts, e=e, pend_g=pend_g):
                    for m, ps, pt in zip(ms, paps, pts):
                        if m % 2 == 0:
                            pend_g[m] = (ps, pt)
                        else:
                            gps, _ = pend_g.pop(m - 1)
                            sg = small.tile([BLK, cap], BF16, tag="sg")
                            nc.scalar.activation(sg[:], gps, AF.Silu)
                            p = m // 2
                            nc.vector.tensor_mul(
                                hm_sb[:, (e * MI + p) * cap:(e * MI + p + 1) * cap],
                                sg[:], ps)

                ws_src = lambda kg0, kgn, c0, c1, e=e: wsT_d[e, kg0 * BLK:(kg0 + kgn) * BLK, c0:c1] \
                    .rearrange("(k p) c -> p k c", p=BLK)

                def w2s_consume(ms, paps, pts, e=e):
                    for m, ps in zip(ms, paps):
                        mo = outp.tile([BLK, cap], F32, tag="mo")
                        nc.vector.tensor_mul(mo[:], ps, ew_sb[:, e * cap:(e + 1) * cap])
                        nc.scalar.dma_start(moe_out_d[e, m * BLK:(m + 1) * BLK, :], mo[:])

                w2s_src = lambda kg0, kgn, c0, c1, e=e: w2sT_d[e, kg0 * BLK:(kg0 + kgn) * BLK, c0:c1] \
                    .rearrange("(k p) c -> p k c", p=BLK)

                for s0 in range(0, 2 * MI, 4):
                    moe_thunks.append(lambda s0=s0, e=e, f=gu_consume, w=ws_src, r=rhs_e: gemm(
                        w, 2 * MI, KQ, r, macc, cap, f, 4, eng_rr, "macct",
                        sweep_starts=[s0]))
                for s0 in range(0, KQ, 4):
                    moe_thunks.append(lambda s0=s0, e=e, f=w2s_consume, w=w2s_src: gemm(
                        w, KQ, MI,
                        lambda k, e=e: hm_sb[:, (e * MI + k) * cap:(e * MI + k + 1) * cap],
                        macc, cap, f, 4, eng_rr, "macct", sweep_starts=[s0]))

            # ---------------- attention (merged q-blocks) ----------------
            def attention_head(h):
                kvh = h // (NH // NKV)
                qv2 = q_sb[:, h * TPC: (h + 1) * TPC]        # both q-blocks [d, 256]
                qv1 = q_sb[:, h * TPC + BLK: (h + 1) * TPC]  # deep block only
                aps = acc.tile([BLK, TPC], F32, tag="acct")  # attn^T [d, 256]
                dps = acc.tile([1, TPC], F32, tag="acct")
                # diagonal pairs (own tokens, static tri mask)
                for qb in range(2):
                    sps = acc.tile([BLK, BLK], F32, tag="acct")
                    nc.tensor.matmul(
                        sps[:], k_sb[:, kvh * TPC + qb * BLK: kvh * TPC + (qb + 1) * BLK],
                        q_sb[:, h * TPC + qb * BLK: h * TPC + (qb + 1) * BLK],
                        start=True, stop=True)
                    stmp = small.tile([BLK, BLK], F32, tag="stmp")
                    nc.vector.tensor_add(stmp[:], sps[:], tri_sb[:])
                    pd = small.tile([BLK, BLK], BF16, tag="pd")
                    nc.scalar.activation(pd[:], stmp[:], AF.Exp, scale=SCALE)
                    nc.tensor.matmul(aps[:, qb * BLK:(qb + 1) * BLK],
                                     v_sb[:, (qb * NKV + kvh) * BLK:(qb * NKV + kvh + 1) * BLK],
                                     pd[:], start=(qb == 0), stop=False)
                    nc.tensor.matmul(dps[:, qb * BLK:(qb + 1) * BLK], ones_bf[:], pd[:],
                                     start=(qb == 0), stop=False)
                # dense blocks: kb<8 merged (both q-blocks), kb>=8 deep only
                for kb in range(NBLK):
                    sl = _slot(kb)
                    kap = kag_sb[:, (sl * NKV + kvh) * BLK:(sl * NKV + kvh + 1) * BLK]
                    vap = vag_sb[:, (sl * NKV + kvh) * BLK:(sl * NKV + kvh + 1) * BLK]
                    merged = kb < NC_
                    wq = TPC if merged else BLK
                    sps2 = acc.tile([BLK, TPC], F32, tag="acct", name="sps2")
                    nc.tensor.matmul(sps2[:, :wq], kap, qv2 if merged else qv1,
                                     start=True, stop=True)
                    pdn = small.tile([BLK, TPC], BF16, tag="pd2")
                    if merged:
                        nc.scalar.activation(
                            pdn[:, 0:BLK], sps2[:, 0:BLK], AF.Exp, scale=SCALE,
                            bias=bias_sb[:, kb: kb + 1])
                        nc.scalar.activation(
                            pdn[:, BLK:TPC], sps2[:, BLK:TPC], AF.Exp, scale=SCALE,
                            bias=bias_sb[:, NBLK + kb: NBLK + kb + 1])
                    else:
                        nc.scalar.activation(
                            pdn[:, 0:BLK], sps2[:, 0:BLK], AF.Exp, scale=SCALE,
                            bias=bias_sb[:, NBLK + kb: NBLK + kb + 1])
                    last = kb == NBLK - 1
                    oap = aps[:] if merged else aps[:, BLK:TPC]
                    dap = dps[:] if merged else dps[:, BLK:TPC]
                    nc.tensor.matmul(oap, vap, pdn[:, :wq], start=False, stop=last)
                    nc.tensor.matmul(dap, ones_bf[:], pdn[:, :wq], start=False, stop=last)
                # normalize
                rec = small.tile([1, TPC], F32, tag="rec")
                nc.vector.reciprocal(rec[:], dps[:])
                bcp = acc.tile([BLK, TPC], F32, tag="acct")
                nc.tensor.matmul(bcp[:], ones_row[:], rec[:], start=True, stop=True)
                bcs = small.tile([BLK, TPC], F32, tag="bcs")
                nc.scalar.activation(bcs[:], bcp[:], AF.Copy)
                nc.vector.tensor_mul(attnT_sb[:, h * TPC:(h + 1) * TPC], aps[:], bcs[:])

            for h in range(NH):
                if h < len(moe_thunks):
                    moe_thunks[h]()
                attention_head(h)
            for th in moe_thunks[NH:]:
                th()

            # ---------------- wo + residual ----------------
            def wo_consume(ms, paps, pts):
                for jt, pt in enumerate(pts):
                    m0 = ms[2 * jt]
                    w = pt.shape[1]
                    nc.vector.tensor_add(resid_sb[:, m0 * TPC: m0 * TPC + w],
                                         pt[:], xT32_sb[:, m0 * TPC: m0 * TPC + w])

            wo_src = lambda kg0, kgn, c0, c1: wo_d[kg0 * BLK:(kg0 + kgn) * BLK, c0:c1] \
                .rearrange("(k p) c -> p k c", p=BLK)
            gemm(wo_src, KQ, KQ, lambda k: attnT_sb[:, k * TPC:(k + 1) * TPC],
                 acc, TPC, wo_consume, SW, eng_rr, "acct")

            # ---------------- residual MLP norm scale ----------------
            ssq = acc.tile([1, TPC], F32, tag="acct")
            for k in range(KQ):
                sq = small.tile([BLK, TPC], F32, tag="sq")
                nc.vector.tensor_mul(sq[:], resid_sb[:, k * TPC:(k + 1) * TPC],
                                     resid_sb[:, k * TPC:(k + 1) * TPC])
                nc.tensor.matmul(ssq[:], ones_cf[:], sq[:],
                                 start=(k == 0), stop=(k == KQ - 1))
            vtmp2 = small.tile([1, TPC], F32, tag="vt")
            nc.vector.tensor_scalar(vtmp2[:], ssq[:], 1.0 / H, EPS,
                                    mybir.AluOpType.mult, mybir.AluOpType.add)
            st = small.tile([1, TPC], F32, tag="vt2")
            nc.scalar.activation(st[:], vtmp2[:], AF.Sqrt)
            s2r = small.tile([1, TPC], F32, tag="vt3")
            nc.vector.reciprocal(s2r[:], st[:])
            s2p = acc.tile([BLK, TPC], F32, tag="acct")
            nc.tensor.matmul(s2p[:], ones_row[:], s2r[:], start=True, stop=True)
            s2s = small.tile([BLK, TPC], F32, tag="s2s")
            nc.scalar.activation(s2s[:], s2p[:], AF.Copy)
            for k in range(KQ):
                nc.vector.tensor_mul(h2_sb[:, k * TPC:(k + 1) * TPC],
                                     resid_sb[:, k * TPC:(k + 1) * TPC], s2s[:])

            # ---------------- w13 (interleaved g/u) + silu_and_mul ----------------
            def w13_consume(ms, paps, pts):
                for jt, pt in enumerate(pts):
                    p = ms[2 * jt] // 2
                    sg = small.tile([BLK, TPC], BF16, tag="sg13")
                    nc.scalar.activation(sg[:], pt[:, 0:TPC], AF.Silu)
                    nc.vector.tensor_mul(gu_sb[:, p * TPC:(p + 1) * TPC],
                                         sg[:], pt[:, TPC:2 * TPC])

            w13_src = lambda kg0, kgn, c0, c1: w13_d[kg0 * BLK:(kg0 + kgn) * BLK, c0:c1] \
                .rearrange("(k p) c -> p k c", p=BLK)
            gemm(w13_src, 2 * KQ, KQ, lambda k: h2_sb[:, k * TPC:(k + 1) * TPC],
                 acc, TPC, w13_consume, SW, eng_rr, "acct")

            # ---------------- w2 + final out ----------------
            def w2_consume(ms, paps, pts):
                for jt, pt in enumerate(pts):
                    m0 = ms[2 * jt]
                    w = pt.shape[1]
                    fo = outp.tile([BLK, 2 * TPC], F32, tag="fo")
                    nc.vector.tensor_add(fo[:, :w], pt[:], resid_sb[:, m0 * TPC:m0 * TPC + w])
                    nc.sync.dma_start(
                        res_out_d.ap().rearrange("(m p) t -> p m t", p=BLK)[:, m0:m0 + w // TPC],
                        fo[:, :w].rearrange("p (m t) -> p m t", t=TPC))

            w2_src = lambda kg0, kgn, c0, c1: w2_d[kg0 * BLK:(kg0 + kgn) * BLK, c0:c1] \
                .rearrange("(k p) c -> p k c", p=BLK)
            gemm(w2_src, KQ, KQ, lambda k: gu_sb[:, k * TPC:(k + 1) * TPC],
                 acc, TPC, w2_consume, SW, eng_rr, "acct")

            if DEBUG_TAPS:
                for nm, sb in [("q", q_sb), ("k", k_sb), ("v", v_sb),
                               ("kag", kag_sb), ("vag", vag_sb), ("attnT", attnT_sb),
                               ("resid", resid_sb), ("h2t", h2_sb), ("gut", gu_sb)]:
                    nc.sync.dma_start(taps[nm].ap(), sb[:])

    nc.compile()
    return nc


def _interleave_cols(w, half):
    # [rows, 2*half] -> column chunks reordered so chunk 2p=g_p, 2p+1=u_p
    rows = w.shape[0]
    g = w[:, :half].reshape(rows, half // BLK, BLK)
    u = w[:, half:].reshape(rows, half // BLK, BLK)
    out = np.empty((rows, 2 * (half // BLK), BLK), w.dtype)
    out[:, 0::2] = g
    out[:, 1::2] = u
    return out.reshape(rows, 2 * half // BLK * BLK)


def kernel(**inputs):
    global LAST_RESULT
    hidden = f32(inputs["hidden_states"])
    positions = np.asarray(inputs["positions"]).astype(np.float32)
    ln_in_w = f32(inputs["ln_in_w"])
    ln_post_w = f32(inputs["ln_post_w"])
    ln_res_w = f32(inputs["ln_res_w"])
    wqkv = f32(inputs["wqkv"])
    wo = f32(inputs["wo"])
    res_w13 = f32(inputs["res_w13"])
    res_w2 = f32(inputs["res_w2"])
    gate_w = f32(inputs["gate_w"])
    ws = f32(inputs["ws"])
    w2s = f32(inputs["w2s"])

    # ---- host prep (sharding) ----
    s = 1.0 / np.sqrt(np.mean(hidden * hidden, axis=1) + EPS)  # [T]
    x_norm = hidden * s[:, None]

    logits = (x_norm * ln_post_w) @ gate_w
    pr = np.exp(logits - logits.max(-1, keepdims=True))
    pr /= pr.sum(-1, keepdims=True)
    topi = np.argsort(-pr, axis=-1, kind="stable")[:, :TOPK]
    topw = np.take_along_axis(pr, topi, axis=-1)
    topw /= topw.sum(-1, keepdims=True)
    tok_lists = [np.where((topi == e).any(-1))[0] for e in range(E)]
    wts = [np.sum(np.where(topi[tl] == e, topw[tl], 0.0), -1).astype(np.float32)
           for e, tl in zip(range(E), tok_lists)]
    cap = max(128, -(-max(len(t) for t in tok_lists) // 64) * 64)
    assert cap <= 512, cap

    ck = (cap, DEBUG_TAPS)
    if ck not in _CACHE:
        _CACHE[ck] = _build(cap)
    nc = _CACHE[ck]

    inv_freq = 1.0 / (THETA ** (np.arange(0, HD, 2, dtype=np.float32) / HD))
    ang = positions[:, None] * inv_freq
    cos_t, sin_t = np.cos(ang), np.sin(ang)

    tri = np.where(np.arange(BLK)[None, :] >= np.arange(BLK)[:, None], 0.0, NEG).astype(np.float32)
    ident = np.eye(BLK, dtype=np.float32)

    wqkv_f = wqkv * ln_in_w[:, None]
    w13_f = _interleave_cols(res_w13 * ln_res_w[:, None], H)
    x_norm_post = x_norm * ln_post_w
    wsT = ws.transpose(0, 2, 1)  # [E, H, 2I]
    wsT_il = np.stack([_interleave_cols(wsT[e], I) for e in range(E)])
    w2sT = w2s.transpose(0, 2, 1)

    shared = {
        "tri": tri, "ident": bf(ident),
        "wqkv": bf(wqkv_f), "wo": bf(wo), "w13": bf(w13_f), "w2": bf(res_w2),
    }

    in_maps = []
    own = [[i, NBLK - 1 - i] for i in range(NC_)]
    for i in range(NC_):
        toks = np.concatenate([np.arange(b * BLK, (b + 1) * BLK) for b in own[i]])
        xT = hidden[toks].T
        cs = (cos_t[toks] * s[toks, None]).T
        sn = (sin_t[toks] * s[toks, None]).T
        scol = np.stack([s[toks[:BLK]], s[toks[BLK:]]], axis=1)
        bias = np.zeros((2, NBLK, BLK), np.float32)
        b0, b1 = own[i]
        bias[0, b0:, :] = NEG
        bias[1, b1:, :] = NEG
        exps = [2 * i, 2 * i + 1]
        xg = np.zeros((EPC, H, cap), np.float32)
        ew = np.zeros((EPC, BLK, cap), np.float32)
        for j, e in enumerate(exps):
            n = len(tok_lists[e])
            xg[j, :, :n] = x_norm_post[tok_lists[e]].T
            ew[j, :, :n] = wts[e][None, :]
        in_maps.append({
            "xT_bf": bf(xT), "xT32": f32(xT),
            "cos_s": f32(cs), "sin_s": f32(sn), "s_col": f32(scol),
            "bias": bias,
            "wsT": bf(wsT_il[exps]),
            "w2sT": bf(w2sT[exps]),
            "xgT": bf(xg), "ew": ew,
            **shared,
        })

    res = run_bass_kernel_spmd(nc, in_maps, core_ids=list(range(NC_)), trace=TRACE)
    LAST_RESULT = res

    out = np.zeros((T, H), np.float32)
    for i in range(NC_):
        toks = np.concatenate([np.arange(b * BLK, (b + 1) * BLK) for b in own[i]])
        out[toks] = res.results[i]["res_out"].T
    for i in range(NC_):
        for j, e in enumerate((2 * i, 2 * i + 1)):
            tl = tok_lists[e]
            out[tl] += res.results[i]["moe_out"][j].T[:len(tl)]
    return out


# revision 22
# speedup vs baseline: 2.0503x; 1.0883x over previous
"""ArcticDecoderLayer on 8 TRN2 NeuronCores.

Sharding:
  - tokens: zigzag block-parallel (core i owns 128-token blocks {i, 15-i});
    attention, wo, residual MLP are token-parallel (weights replicated).
  - MoE: expert-parallel, 2 experts/core; routing/top-2 + token gather/scatter
    done host-side (part of shard/unshard), expert GEMMs on device.
  - One AllGather (K^T feature-major + V token-major, bf16) is the only
    collective; causal masking is data-driven (per-core exp-bias columns) so
    the SPMD graph is identical on all cores.

All matmuls in bf16 (fp32 PE matmul is 4x slower); accumulation in f32 PSUM.
Weight streaming uses few large multi-k-chunk DMAs (3D APs) split across the
two HWDGE rings (sync + scalar) to keep sequencer issue cost off the critical
path.
"""
import numpy as np
import ml_dtypes

import concourse.bacc as bacc
import concourse.tile as tile
import concourse.mybir as mybir
from concourse.bass_utils import run_bass_kernel_spmd

F32 = mybir.dt.float32
BF16 = mybir.dt.bfloat16
AF = mybir.ActivationFunctionType

H = 2048
NH = 16
NKV = 4
HD = 128
HALF = 64
I = 1024
E = 16
TOPK = 2
T = 2048
EPS = 1e-5
THETA = 10000.0
NC_ = 8
BLK = 128
NBLK = 16
TPC = 256  # tokens per core
EPC = 2  # experts per core
SCALE = HD ** -0.5
NEG = -30000.0
KQ = H // BLK  # 16
MI = I // BLK  # 8

TRACE = False
DEBUG_TAPS = False
LAST_RESULT = None
_CACHE = {}

bf = lambda a: np.ascontiguousarray(np.asarray(a).astype(ml_dtypes.bfloat16))
f32 = lambda a: np.ascontiguousarray(a, dtype=np.float32)


def _slot(kb):
    c = min(kb, NBLK - 1 - kb)
    return 2 * c + (0 if kb < NC_ else 1)


def _build(cap):
    nc = bacc.Bacc("TRN2", target_bir_lowering=False, debug=False, num_devices=NC_)

    din = lambda name, shape, dt=BF16: nc.dram_tensor(name, shape, dt, kind="ExternalInput")
    xT_bf_d = din("xT_bf", [H, TPC])
    xT32_d = din("xT32", [H, TPC], F32)
    cos_d = din("cos_s", [HALF, TPC], F32)
    sin_d = din("sin_s", [HALF, TPC], F32)
    scol_d = din("s_col", [BLK, 2], F32)
    bias_d = din("bias", [2, NBLK, BLK], F32)
    tri_d = din("tri", [BLK, BLK], F32)
    ident_d = din("ident", [BLK, BLK])
    wqkv_d = din("wqkv", [H, NH * HD + 2 * NKV * HD])
    wo_d = din("wo", [NH * HD, H])
    w13_d = din("w13", [H, 2 * H])  # host-interleaved: chunk 2p=g_p, 2p+1=u_p
    w2_d = din("w2", [H, H])
    wsT_d = din("wsT", [EPC, H, 2 * I])  # host-interleaved g/u pairs
    w2sT_d = din("w2sT", [EPC, I, H])
    xg_d = din("xgT", [EPC, H, cap])
    ew_d = din("ew", [EPC, BLK, cap], F32)

    res_out_d = nc.dram_tensor("res_out", [H, TPC], F32, kind="ExternalOutput")
    moe_out_d = nc.dram_tensor("moe_out", [EPC, H, cap], F32, kind="ExternalOutput")
    taps = {}
    if DEBUG_TAPS:
        for nm, w in [("q", NH * TPC), ("k", NKV * TPC), ("v", 2 * NKV * HD),
                      ("kag", NBLK * NKV * BLK), ("vag", NBLK * NKV * BLK),
                      ("attnT", NH * TPC), ("resid", KQ * TPC), ("h2t", KQ * TPC),
                      ("gut", KQ * TPC)]:
            taps[nm] = nc.dram_tensor("tap_" + nm, [BLK, w],
                                      F32 if nm == "resid" else BF16,
                                      kind="ExternalOutput")

    KG = 4  # contraction chunks per weight-stream DMA
    SW = 8  # m-chunks per sweep for TN=256 GEMMs (4 paired psum banks)

    with tile.TileContext(nc) as tc:
        with (
            tc.tile_pool(name="res", bufs=1) as res,
            tc.tile_pool(name="stream", bufs=2) as stream,
            tc.tile_pool(name="small", bufs=2) as small,
            tc.tile_pool(name="outp", bufs=2) as outp,
            tc.tile_pool(name="acc", bufs=4, space="PSUM") as acc,
            tc.tile_pool(name="macc", bufs=4, space="PSUM") as macc,
            tc.tile_pool(name="dram", bufs=1, space="DRAM") as dram,
        ):
            eng_rr = [nc.sync, nc.scalar]

            # ---------------- resident loads ----------------
            xT_sb = res.tile([BLK, KQ * TPC], BF16, tag="xT")
            xT32_sb = res.tile([BLK, KQ * TPC], F32, tag="xT32")
            nc.sync.dma_start(xT_sb[:].rearrange("p (k t) -> p k t", k=KQ),
                              xT_bf_d.ap().rearrange("(k p) t -> p k t", p=BLK))
            cos2_sb = res.tile([HALF, 2 * TPC], F32, tag="cos")
            sin2_sb = res.tile([HALF, 2 * TPC], F32, tag="sin")
            for _rep in range(2):
                nc.sync.dma_start(cos2_sb[:, _rep * TPC:(_rep + 1) * TPC], cos_d[:])
                nc.sync.dma_start(sin2_sb[:, _rep * TPC:(_rep + 1) * TPC], sin_d[:])
            scol_sb = res.tile([BLK, 2], F32, tag="scol")
            nc.sync.dma_start(scol_sb[:], scol_d[:])
            bias_sb = res.tile([BLK, 2 * NBLK], F32, tag="bias")
            nc.sync.dma_start(bias_sb[:], bias_d.ap().rearrange("a k p -> p (a k)"))
            tri_sb = res.tile([BLK, BLK], F32, tag="tri")
            nc.sync.dma_start(tri_sb[:], tri_d[:])
            ident_sb = res.tile([BLK, BLK], BF16, tag="ident")
            nc.sync.dma_start(ident_sb[:], ident_d[:])
            ew_sb = res.tile([BLK, EPC * cap], F32, tag="ew")
            xg_sb = res.tile([BLK, EPC * KQ * cap], BF16, tag="xg")

            ones_bf = res.tile([BLK, 1], BF16, tag="onesb")
            nc.vector.memset(ones_bf[:], 1.0)
            ones_row = res.tile([1, BLK], F32, tag="onesr")
            nc.vector.memset(ones_row[:], 1.0)
            ones_cf = res.tile([BLK, 1], F32, tag="onescf")
            nc.vector.memset(ones_cf[:], 1.0)
            ones16 = res.tile([NH, BLK], F32, tag="ones16")
            nc.vector.memset(ones16[:], 1.0)
            den_sb = res.tile([BLK, 4 * TPC], F32, tag="densb")
            nc.vector.memset(den_sb[:], 1.0)
            rec_sb = res.tile([BLK, 4 * TPC], F32, tag="recsb")
            attnU_sb = res.tile([BLK, NH * TPC], BF16, tag="attnU")

            q_sb = res.tile([BLK, NH * TPC], BF16, tag="q")
            k_sb = res.tile([BLK, NKV * TPC], BF16, tag="k")
            v_sb = res.tile([BLK, 2 * NKV * HD], BF16, tag="v")
            resid_sb = res.tile([BLK, KQ * TPC], F32, tag="resid")
            h2_sb = res.tile([BLK, KQ * TPC], BF16, tag="h2")
            gu_sb = res.tile([BLK, KQ * TPC], BF16, tag="gu")
            hm_sb = res.tile([BLK, EPC * MI * cap], BF16, tag="hm")
            kag_sb = res.tile([BLK, NBLK * NKV * BLK], BF16, tag="kag")
            vag_sb = res.tile([BLK, NBLK * NKV * BLK], BF16, tag="vag")

            # ============ generic streamed GEMM sweep ============
            # out chunk m = w[:, m*128:(m+1)*128].T @ rhs ; accumulate over kcnt
            # contraction chunks. consume(m, ps_ap) or consume_pair(j, ps, ms).
            def gemm(w_src, mcnt, kcnt, rhs_fn, pool, tn, consume, sweep, dma_eng, tag,
                     sweep_starts=None):
                pair = 2 * tn <= 512
                for s0 in (sweep_starts if sweep_starts is not None
                           else range(0, mcnt, sweep)):
                    ms = list(range(s0, min(s0 + sweep, mcnt)))
                    mw = len(ms)
                    if pair:
                        nt = (mw + 1) // 2
                        pts = [pool.tile([BLK, 2 * tn], F32, tag=tag,
                                         name=f"pt{j}") for j in range(nt)]
                        paps = [pts[j // 2][:, (j % 2) * tn:(j % 2 + 1) * tn]
                                for j in range(mw)]
                    else:
                        pts = [pool.tile([BLK, tn], F32, tag=tag,
                                         name=f"pt{j}") for j in range(mw)]
                        paps = [pts[j][:] for j in range(mw)]
                    for kg0 in range(0, kcnt, KG):
                        kgn = min(KG, kcnt - kg0)
                        wt = stream.tile([BLK, KG * SW * BLK], BF16, tag="wt")
                        dma_eng[(kg0 // KG) % 2].dma_start(
                            wt[:, :kgn * mw * BLK].rearrange("p (k c) -> p k c", k=kgn),
                            w_src(kg0, kgn, ms[0] * BLK, (ms[-1] + 1) * BLK))
                        for kl in range(kgn):
                            k = kg0 + kl
                            for j in range(mw):
                                # start=True clears the WHOLE psum bank, so for
                                # paired chunks only the first chunk of a tile
                                # may set it (partner then overwrites via
                                # cleared has_written bits).
                                first = (j % 2 == 0) if pair else True
                                last = (j % 2 == 1 or j == mw - 1) if pair else True
                                nc.tensor.matmul(
                                    paps[j], wt[:, (kl * mw + j) * BLK:(kl * mw + j + 1) * BLK],
                                    rhs_fn(k), start=(k == 0 and first),
                                    stop=(k == kcnt - 1 and last))
                    consume(ms, paps, pts)

            # ---------------- QKV projection (feature-major out) ----------------
            def rope_pair(pt2, dst, col0):
                # pt2: [128, 512] psum pair (two head-chunks side by side)
                t1 = small.tile([HALF, 2 * TPC], F32, tag="r1")
                t2 = small.tile([HALF, 2 * TPC], F32, tag="r2")
                nc.vector.tensor_mul(t1[:], pt2[0:HALF, :], cos2_sb[:])
                nc.vector.tensor_mul(t2[:], pt2[HALF:BLK, :], sin2_sb[:])
                nc.vector.tensor_sub(dst[0:HALF, col0:col0 + 2 * TPC], t1[:], t2[:])
                t3 = small.tile([HALF, 2 * TPC], F32, tag="r1")
                t4 = small.tile([HALF, 2 * TPC], F32, tag="r2")
                nc.vector.tensor_mul(t3[:], pt2[HALF:BLK, :], cos2_sb[:])
                nc.vector.tensor_mul(t4[:], pt2[0:HALF, :], sin2_sb[:])
                nc.vector.tensor_add(dst[HALF:BLK, col0:col0 + 2 * TPC], t3[:], t4[:])

            def qkv_consume(ms, paps, pts):
                for jt, pt2 in enumerate(pts):
                    m = ms[2 * jt]
                    if m < NH:
                        rope_pair(pt2[:], q_sb, m * TPC)
                    elif m < NH + NKV:
                        rope_pair(pt2[:], k_sb, (m - NH) * TPC)
                    else:
                        for half_j in range(2):
                            kvh = m + half_j - NH - NKV
                            ps = pt2[:, half_j * TPC:(half_j + 1) * TPC]
                            vtmp = small.tile([BLK, TPC], BF16, tag="vtmp")
                            nc.scalar.activation(vtmp[:], ps, AF.Copy)
                            for tb in range(2):
                                ptt = macc.tile([BLK, BLK], BF16, tag="macct")
                                nc.tensor.transpose(ptt[:], vtmp[:, tb * BLK:(tb + 1) * BLK], ident_sb[:])
                                nc.vector.tensor_scalar_mul(
                                    v_sb[:, (tb * NKV + kvh) * BLK:(tb * NKV + kvh + 1) * BLK],
                                    ptt[:], scol_sb[:, tb:tb + 1])

            wqkv_src = lambda kg0, kgn, c0, c1: wqkv_d[kg0 * BLK:(kg0 + kgn) * BLK, c0:c1] \
                .rearrange("(k p) c -> p k c", p=BLK)
            # KV chunks first so the AllGather can launch early
            gemm(wqkv_src, NH + 2 * NKV, KQ,
                 lambda k: xT_sb[:, k * TPC:(k + 1) * TPC], acc, TPC, qkv_consume,
                 SW, eng_rr, "acct", sweep_starts=[16])

            # ---------------- KV AllGather ----------------
            KSZ = NKV * BLK * TPC  # 131072 elems
            kv_local = dram.tile([2, KSZ], BF16)
            kv_ag = dram.tile([NC_, 2, KSZ], BF16, addr_space="Shared")
            nc.sync.dma_start(
                kv_local[0, :].rearrange("(h d t) -> d h t", h=NKV, d=BLK),
                k_sb[:].rearrange("d (h t) -> d h t", h=NKV))
            nc.sync.dma_start(
                kv_local[1, :].rearrange("(b h t d) -> t b h d", b=2, h=NKV, t=BLK),
                v_sb[:].rearrange("t (b h d) -> t b h d", b=2, h=NKV))
            nc.gpsimd.collective_compute(
                "AllGather", mybir.AluOpType.bypass,
                replica_groups=[list(range(NC_))],
                ins=[kv_local[:]], outs=[kv_ag[:]])
            # remaining qkv sweeps (q heads) overlap the collective
            gemm(wqkv_src, NH + 2 * NKV, KQ,
                 lambda k: xT_sb[:, k * TPC:(k + 1) * TPC], acc, TPC, qkv_consume,
                 SW, eng_rr, "acct", sweep_starts=[0, 8])
            # deferred resident loads (needed by MoE / wo, not by qkv)
            for e in range(EPC):
                nc.scalar.dma_start(
                    xg_sb[:, e * KQ * cap:(e + 1) * KQ * cap].rearrange("p (k t) -> p k t", k=KQ),
                    xg_d[e].rearrange("(k p) t -> p k t", p=BLK))
                nc.scalar.dma_start(ew_sb[:, e * cap:(e + 1) * cap], ew_d[e])
            nc.scalar.dma_start(xT32_sb[:].rearrange("p (k t) -> p k t", k=KQ),
                                xT32_d.ap().rearrange("(k p) t -> p k t", p=BLK))
            # unpack in AG order (slot = 2c+sub); attention maps kb -> slot.
            for sub in range(2):
                for hh in range(NKV):
                    nc.sync.dma_start(
                        kag_sb[:].rearrange("d (c s h t) -> s h d c t", c=NC_, s=2, h=NKV)[sub, hh],
                        kv_ag[:, 0, :].rearrange("c (h d s t) -> s h d c t", h=NKV, d=BLK, s=2)[sub, hh])
                    nc.scalar.dma_start(
                        vag_sb[:].rearrange("t (c s h dd) -> s h t c dd", c=NC_, s=2, h=NKV)[sub, hh],
                        kv_ag[:, 1, :].rearrange("c (s h t dd) -> s h t c dd", s=2, h=NKV, t=BLK)[sub, hh])

            # ------- MoE sweeps (as thunks) interleaved with attention heads -------
            moe_thunks = []
            for e in range(EPC):
                rhs_e = lambda k, e=e: xg_sb[:, (e * KQ + k) * cap:(e * KQ + k + 1) * cap]
                pend_g = {}

                def gu_consume(ms, paps, pts, e=e, pend_g=pend_g):
                    for m, ps, pt in zip(ms, paps, pts):
                        if m % 2 == 0:
                            pend_g[m] = (ps, pt)
                        else:
                            gps, _ = pend_g.pop(m - 1)
                            sg = small.tile([BLK, cap], BF16, tag="sg")
                            nc.scalar.activation(sg[:], gps, AF.Silu)
                            p = m // 2
                            nc.vector.tensor_mul(
                                hm_sb[:, (e * MI + p) * cap:(e * MI + p + 1) * cap],
                                sg[:], ps)

                ws_src = lambda kg0, kgn, c0, c1, e=e: wsT_d[e, kg0 * BLK:(kg0 + kgn) * BLK, c0:c1] \
                    .rearrange("(k p) c -> p k c", p=BLK)

                def w2s_consume(ms, paps, pts, e=e):
                    for m, ps in zip(ms, paps):
                        mo = outp.tile([BLK, cap], F32, tag="mo")
                        nc.vector.tensor_mul(mo[:], ps, ew_sb[:, e * cap:(e + 1) * cap])
                        nc.scalar.dma_start(moe_out_d[e, m * BLK:(m + 1) * BLK, :], mo[:])

                w2s_src = lambda kg0, kgn, c0, c1, e=e: w2sT_d[e, kg0 * BLK:(kg0 + kgn) * BLK, c0:c1] \
                    .rearrange("(k p) c -> p k c", p=BLK)

                for s0 in range(0, 2 * MI, 4):
                    moe_thunks.append(lambda s0=s0, e=e, f=gu_consume, w=ws_src, r=rhs_e: gemm(
                        w, 2 * MI, KQ, r, macc, cap, f, 4, eng_rr, "macct",
                        sweep_starts=[s0]))
                for s0 in range(0, KQ, 4):
                    moe_thunks.append(lambda s0=s0, e=e, f=w2s_consume, w=w2s_src: gemm(
                        w, KQ, MI,
                        lambda k, e=e: hm_sb[:, (e * MI + k) * cap:(e * MI + k + 1) * cap],
                        macc, cap, f, 4, eng_rr, "macct", sweep_starts=[s0]))

            # ---------------- attention (merged q-blocks) ----------------
            def attention_head(h):
                kvh = h // (NH // NKV)
                qv2 = q_sb[:, h * TPC: (h + 1) * TPC]        # both q-blocks [d, 256]
                qv1 = q_sb[:, h * TPC + BLK: (h + 1) * TPC]  # deep block only
                aps = acc.tile([BLK, TPC], F32, tag="acct")  # attn^T [d, 256]
                dps = acc.tile([1, TPC], F32, tag="acct")
                # diagonal pairs (own tokens, static tri mask)
                for qb in range(2):
                    sps = acc.tile([BLK, BLK], F32, tag="acct")
                    nc.tensor.matmul(
                        sps[:], k_sb[:, kvh * TPC + qb * BLK: kvh * TPC + (qb + 1) * BLK],
                        q_sb[:, h * TPC + qb * BLK: h * TPC + (qb + 1) * BLK],
                        start=True, stop=True)
                    stmp = small.tile([BLK, BLK], F32, tag="stmp")
                    nc.vector.tensor_add(stmp[:], sps[:], tri_sb[:])
                    pd = small.tile([BLK, BLK], BF16, tag="pd")
                    nc.scalar.activation(pd[:], stmp[:], AF.Exp, scale=SCALE)
                    nc.tensor.matmul(aps[:, qb * BLK:(qb + 1) * BLK],
                                     v_sb[:, (qb * NKV + kvh) * BLK:(qb * NKV + kvh + 1) * BLK],
                                     pd[:], start=(qb == 0), stop=False)
                    nc.tensor.matmul(dps[:, qb * BLK:(qb + 1) * BLK], ones_bf[:], pd[:],
                                     start=(qb == 0), stop=False)
                # dense blocks: kb<8 merged (both q-blocks), kb>=8 deep only
                for kb in range(NBLK):
                    sl = _slot(kb)
                    kap = kag_sb[:, (sl * NKV + kvh) * BLK:(sl * NKV + kvh + 1) * BLK]
                    vap = vag_sb[:, (sl * NKV + kvh) * BLK:(sl * NKV + kvh + 1) * BLK]
                    merged = kb < NC_
                    wq = TPC if merged else BLK
                    sps2 = acc.tile([BLK, TPC], F32, tag="acct", name="sps2")
                    nc.tensor.matmul(sps2[:, :wq], kap, qv2 if merged else qv1,
                                     start=True, stop=True)
                    pdn = small.tile([BLK, TPC], BF16, tag="pd2")
                    if merged:
                        nc.scalar.activation(
                            pdn[:, 0:BLK], sps2[:, 0:BLK], AF.Exp, scale=SCALE,
                            bias=bias_sb[:, kb: kb + 1])
                        nc.scalar.activation(
                            pdn[:, BLK:TPC], sps2[:, BLK:TPC], AF.Exp, scale=SCALE,
                            bias=bias_sb[:, NBLK + kb: NBLK + kb + 1])
                    else:
                        nc.scalar.activation(
                            pdn[:, 0:BLK], sps2[:, 0:BLK], AF.Exp, scale=SCALE,
                            bias=bias_sb[:, NBLK + kb: NBLK + kb + 1])
                    last = kb == NBLK - 1
                    oap = aps[:] if merged else aps[:, BLK:TPC]
                    dap = dps[:] if merged else dps[:, BLK:TPC]
                    nc.tensor.matmul(oap, vap, pdn[:, :wq], start=False, stop=last)
                    nc.tensor.matmul(dap, ones_bf[:], pdn[:, :wq], start=False, stop=last)
                # drain unnormalized; stash denominator row for batched recip
                pb, cb = 32 * (h % 4), (h // 4) * TPC
                nc.vector.tensor_copy(den_sb[pb:pb + 1, cb:cb + TPC], dps[:])
                nc.vector.tensor_copy(attnU_sb[:, h * TPC:(h + 1) * TPC], aps[:])

            def normalize_heads(h0, h1):
                g = h0 // 4
                nc.vector.reciprocal(rec_sb[:, g * TPC:(g + 1) * TPC],
                                     den_sb[:, g * TPC:(g + 1) * TPC])
                for h in range(h0, h1):
                    pb, cb = 32 * (h % 4), (h // 4) * TPC
                    rec0 = small.tile([1, TPC], F32, tag="rec0")
                    nc.vector.tensor_copy(rec0[:], rec_sb[pb:pb + 1, cb:cb + TPC])
                    bcp = acc.tile([BLK, TPC], F32, tag="acct")
                    nc.tensor.matmul(bcp[:], ones_row[:], rec0[:],
                                     start=True, stop=True)
                    bcs = small.tile([BLK, TPC], F32, tag="bcs")
                    nc.vector.tensor_copy(bcs[:], bcp[:])
                    nc.vector.tensor_mul(attnU_sb[:, h * TPC:(h + 1) * TPC],
                                         attnU_sb[:, h * TPC:(h + 1) * TPC], bcs[:])

            # front-load a few MoE sweeps to cover the AllGather latency
            nfront = 3
            for th in moe_thunks[:nfront]:
                th()
            rest = moe_thunks[nfront:]
            for h in range(NH):
                if h < len(rest):
                    rest[h]()
                attention_head(h)
                if h % 4 == 3:
                    normalize_heads(h - 3, h + 1)
            for th in rest[NH:]:
                th()

            # ---------------- wo + residual ----------------
            def wo_consume(ms, paps, pts):
                for jt, pt in enumerate(pts):
                    m0 = ms[2 * jt]
                    w = pt.shape[1]
                    nc.vector.tensor_add(resid_sb[:, m0 * TPC: m0 * TPC + w],
                                         pt[:], xT32_sb[:, m0 * TPC: m0 * TPC + w])

            wo_src = lambda kg0, kgn, c0, c1: wo_d[kg0 * BLK:(kg0 + kgn) * BLK, c0:c1] \
                .rearrange("(k p) c -> p k c", p=BLK)
            gemm(wo_src, KQ, KQ, lambda k: attnU_sb[:, k * TPC:(k + 1) * TPC],
                 acc, TPC, wo_consume, SW, eng_rr, "acct")

            # ---------------- residual MLP norm scale ----------------
            ssq = acc.tile([1, TPC], F32, tag="acct")
            for k in range(KQ):
                sq = small.tile([BLK, TPC], F32, tag="sq")
                nc.vector.tensor_mul(sq[:], resid_sb[:, k * TPC:(k + 1) * TPC],
                                     resid_sb[:, k * TPC:(k + 1) * TPC])
                nc.tensor.matmul(ssq[:], ones_cf[:], sq[:],
                                 start=(k == 0), stop=(k == KQ - 1))
            vtmp2 = small.tile([1, TPC], F32, tag="vt")
            nc.vector.tensor_scalar(vtmp2[:], ssq[:], 1.0 / H, EPS,
                                    mybir.AluOpType.mult, mybir.AluOpType.add)
            st = small.tile([1, TPC], F32, tag="vt2")
            nc.scalar.activation(st[:], vtmp2[:], AF.Sqrt)
            s2r = small.tile([1, TPC], F32, tag="vt3")
            nc.vector.reciprocal(s2r[:], st[:])
            s2p = acc.tile([BLK, TPC], F32, tag="acct")
            nc.tensor.matmul(s2p[:], ones_row[:], s2r[:], start=True, stop=True)
            s2s = small.tile([BLK, TPC], F32, tag="s2s")
            nc.scalar.activation(s2s[:], s2p[:], AF.Copy)
            for k in range(KQ):
                nc.vector.tensor_mul(h2_sb[:, k * TPC:(k + 1) * TPC],
                                     resid_sb[:, k * TPC:(k + 1) * TPC], s2s[:])

            # ---------------- w13 (interleaved g/u) + silu_and_mul ----------------
            def w13_consume(ms, paps, pts):
                for jt, pt in enumerate(pts):
                    p = ms[2 * jt] // 2
                    sg = small.tile([BLK, TPC], BF16, tag="sg13")
                    nc.scalar.activation(sg[:], pt[:, 0:TPC], AF.Silu)
                    nc.vector.tensor_mul(gu_sb[:, p * TPC:(p + 1) * TPC],
                                         sg[:], pt[:, TPC:2 * TPC])

            w13_src = lambda kg0, kgn, c0, c1: w13_d[kg0 * BLK:(kg0 + kgn) * BLK, c0:c1] \
                .rearrange("(k p) c -> p k c", p=BLK)
            gemm(w13_src, 2 * KQ, KQ, lambda k: h2_sb[:, k * TPC:(k + 1) * TPC],
                 acc, TPC, w13_consume, SW, eng_rr, "acct")

            # ---------------- w2 + final out ----------------
            def w2_consume(ms, paps, pts):
                for jt, pt in enumerate(pts):
                    m0 = ms[2 * jt]
                    w = pt.shape[1]
                    fo = outp.tile([BLK, 2 * TPC], F32, tag="fo")
                    nc.vector.tensor_add(fo[:, :w], pt[:], resid_sb[:, m0 * TPC:m0 * TPC + w])
                    nc.sync.dma_start(
                        res_out_d.ap().rearrange("(m p) t -> p m t", p=BLK)[:, m0:m0 + w // TPC],
                        fo[:, :w].rearrange("p (m t) -> p m t", t=TPC))

            w2_src = lambda kg0, kgn, c0, c1: w2_d[kg0 * BLK:(kg0 + kgn) * BLK, c0:c1] \
                .rearrange("(k p) c -> p k c", p=BLK)
            gemm(w2_src, KQ, KQ, lambda k: gu_sb[:, k * TPC:(k + 1) * TPC],
                 acc, TPC, w2_consume, SW, eng_rr, "acct")

            if DEBUG_TAPS:
                for nm, sb in [("q", q_sb), ("k", k_sb), ("v", v_sb),
                               ("kag", kag_sb), ("vag", vag_sb), ("attnT", attnT_sb),
                               ("resid", resid_sb), ("h2t", h2_sb), ("gut", gu_sb)]:
                    nc.sync.dma_start(taps[nm].ap(), sb[:])

    nc.compile()
    return nc


def _interleave_cols(w, half):
    # [rows, 2*half] -> column chunks reordered so chunk 2p=g_p, 2p+1=u_p
    rows = w.shape[0]
    g = w[:, :half].reshape(rows, half // BLK, BLK)
    u = w[:, half:].reshape(rows, half // BLK, BLK)
    out = np.empty((rows, 2 * (half // BLK), BLK), w.dtype)
    out[:, 0::2] = g
    out[:, 1::2] = u
    return out.reshape(rows, 2 * half // BLK * BLK)


def kernel(**inputs):
    global LAST_RESULT
    hidden = f32(inputs["hidden_states"])
    positions = np.asarray(inputs["positions"]).astype(np.float32)
    ln_in_w = f32(inputs["ln_in_w"])
    ln_post_w = f32(inputs["ln_post_w"])
    ln_res_w = f32(inputs["ln_res_w"])
    wqkv = f32(inputs["wqkv"])
    wo = f32(inputs["wo"])
    res_w13 = f32(inputs["res_w13"])
    res_w2 = f32(inputs["res_w2"])
    gate_w = f32(inputs["gate_w"])
    ws = f32(inputs["ws"])
    w2s = f32(inputs["w2s"])

    # ---- host prep (sharding) ----
    s = 1.0 / np.sqrt(np.mean(hidden * hidden, axis=1) + EPS)  # [T]
    x_norm = hidden * s[:, None]

    logits = (x_norm * ln_post_w) @ gate_w
    pr = np.exp(logits - logits.max(-1, keepdims=True))
    pr /= pr.sum(-1, keepdims=True)
    topi = np.argsort(-pr, axis=-1, kind="stable")[:, :TOPK]
    topw = np.take_along_axis(pr, topi, axis=-1)
    topw /= topw.sum(-1, keepdims=True)
    tok_lists = [np.where((topi == e).any(-1))[0] for e in range(E)]
    wts = [np.sum(np.where(topi[tl] == e, topw[tl], 0.0), -1).astype(np.float32)
           for e, tl in zip(range(E), tok_lists)]
    cap = max(128, -(-max(len(t) for t in tok_lists) // 64) * 64)
    assert cap <= 512, cap

    ck = (cap, DEBUG_TAPS)
    if ck not in _CACHE:
        _CACHE[ck] = _build(cap)
    nc = _CACHE[ck]

    inv_freq = 1.0 / (THETA ** (np.arange(0, HD, 2, dtype=np.float32) / HD))
    ang = positions[:, None] * inv_freq
    cos_t, sin_t = np.cos(ang), np.sin(ang)

    tri = np.where(np.arange(BLK)[None, :] >= np.arange(BLK)[:, None], 0.0, NEG).astype(np.float32)
    ident = np.eye(BLK, dtype=np.float32)

    wqkv_f = wqkv * ln_in_w[:, None]
    w13_f = _interleave_cols(res_w13 * ln_res_w[:, None], H)
    x_norm_post = x_norm * ln_post_w
    wsT = ws.transpose(0, 2, 1)  # [E, H, 2I]
    wsT_il = np.stack([_interleave_cols(wsT[e], I) for e in range(E)])
    w2sT = w2s.transpose(0, 2, 1)

    shared = {
        "tri": tri, "ident": bf(ident),
        "wqkv": bf(wqkv_f), "wo": bf(wo), "w13": bf(w13_f), "w2": bf(res_w2),
    }

    in_maps = []
    own = [[i, NBLK - 1 - i] for i in range(NC_)]
    for i in range(NC_):
        toks = np.concatenate([np.arange(b * BLK, (b + 1) * BLK) for b in own[i]])
        xT = hidden[toks].T
        cs = (cos_t[toks] * s[toks, None]).T
        sn = (sin_t[toks] * s[toks, None]).T
        scol = np.stack([s[toks[:BLK]], s[toks[BLK:]]], axis=1)
        bias = np.zeros((2, NBLK, BLK), np.float32)
        b0, b1 = own[i]
        bias[0, b0:, :] = NEG
        bias[1, b1:, :] = NEG
        exps = [2 * i, 2 * i + 1]
        xg = np.zeros((EPC, H, cap), np.float32)
        ew = np.zeros((EPC, BLK, cap), np.float32)
        for j, e in enumerate(exps):
            n = len(tok_lists[e])
            xg[j, :, :n] = x_norm_post[tok_lists[e]].T
            ew[j, :, :n] = wts[e][None, :]
        in_maps.append({
            "xT_bf": bf(xT), "xT32": f32(xT),
            "cos_s": f32(cs), "sin_s": f32(sn), "s_col": f32(scol),
            "bias": bias,
            "wsT": bf(wsT_il[exps]),
            "w2sT": bf(w2sT[exps]),
            "xgT": bf(xg), "ew": ew,
            **shared,
        })

    res = run_bass_kernel_spmd(nc, in_maps, core_ids=list(range(NC_)), trace=TRACE)
    LAST_RESULT = res

    out = np.zeros((T, H), np.float32)
    for i in range(NC_):
        toks = np.concatenate([np.arange(b * BLK, (b + 1) * BLK) for b in own[i]])
        out[toks] = res.results[i]["res_out"].T
    for i in range(NC_):
        for j, e in enumerate((2 * i, 2 * i + 1)):
            tl = tok_lists[e]
            out[tl] += res.results[i]["moe_out"][j].T[:len(tl)]
    return out
